# revision 46
# baseline (speedup 1.0000x reference)
"""Trainium2 Bass kernel for nn_FIB_RNN (GRU encoder + autoregressive
sampling decoder with DenseVariational head).

Contract: kernel(**inputs) takes the FULL unsharded inputs (numpy arrays,
keys as in reference.setup_inputs()) and returns the FULL output
[B, GAMMA, 2] float32.

Strategy: pure data parallelism over the batch dim across 8 NeuronCores
(1024 batch rows per core).  The GRU state is feature-major
[U=128 partitions, batch free]; the recurrent matmul is
lhsT=R_gate[128,128] @ rhs=h[128,CW] -> PSUM, with the rank-1 K@x / K@y
input terms accumulated into the same banks.  fp16 end-to-end.

Decoder sample path (v3): the dense head runs BATCH-MAJOR -- for each
128-batch slice, lhsT = h-slice [U,128] (stationary), rhs = the step's
[W0|W1] columns [U,2], landing loc/v as [128, 2*NQ] PSUM with batch on
partitions.  The whole softplus/sample pipeline is then micro-ops:
sigmoid on [128,8] (the sigmoid table stays resident -- ZERO act-table
loads in the entire kernel), ln via exponent-extraction fastlog on the
vector engine (bitcast + shift/mask + deg-3 poly), reparameterized
sample, and one small DMA to flip y back to row form for the K@y
matmuls.  loc/scale outputs stream batch-major to DRAM and the host
unpermutes.
"""

import os
import sys
from contextlib import ExitStack

import numpy as np

for _p in ("/opt/trn_rl_repo", "/root/.axon_site/_ro/trn_rl_repo"):
    if os.path.isdir(_p) and _p not in sys.path:
        sys.path.insert(0, _p)

import concourse.bass as bass
import concourse.tile as tile
from concourse import bacc, mybir
from concourse.bass_utils import run_bass_kernel_spmd
from concourse.dve_ops import AFFINE_MUL_REDUCE

F32 = mybir.dt.float32
U32 = mybir.dt.uint32
I32 = mybir.dt.int32
AF = mybir.ActivationFunctionType
ALU = mybir.AluOpType

U = 128                    # rnn units
T_ENC = 48                 # encoder steps
GAMMA = 28                 # decoder outputs (27 sampled feedback steps)
N_CORES = 8
B_FULL = 8192
BC = B_FULL // N_CORES     # 1024 batch rows per core
CW = int(os.environ.get("KERNEL_CW", "512"))
NCH = BC // CW             # chunks per core
SL = CW // U               # 128-batch slices per chunk
NQ = BC // U               # slices per core (= NCH*SL)
C_SP = float(np.log(np.expm1(1.0)))  # softplus^-1(1.0)
Q_SCALE = 0.02
OP_SCALE = 0.05

# fastlog: ln(f) on [1,2) as c3 f^3 + c2 f^2 + c1 f + c0 (least squares,
# max err ~1e-4); ln(g) = (e_bits - 127)*ln2 + poly(mantissa)
_FS = np.linspace(1.0, 2.0, 8193)
_C3, _C2, _C1, _C0 = [float(v) for v in np.polyfit(_FS, np.log(_FS), 3)]
_LN2 = float(np.log(2.0))
_KLN = _C0 - 127.0 * _LN2          # ln(g) = p4 + _KLN given p4 below
_SAMP_S1 = 1e-5 - OP_SCALE * _KLN  # (1e-5 + 0.05*sp) = _SAMP_S1 - 0.05*p4

_MM_MODE = os.environ.get("KERNEL_MM_DT", "f16")
RD = {
    "f16": mybir.dt.float16,
    "f32r": mybir.dt.float32r,
    "bf16": mybir.dt.bfloat16,
    "f32": F32,
}[_MM_MODE]
RD16 = {
    "f16": mybir.dt.float16,
    "f32r": F32,
    "bf16": mybir.dt.bfloat16,
    "f32": F32,
}[_MM_MODE]

# PE filler matmuls per step (scratch PSUM, no readers): keep the
# tensor engine streaming through its dependency stalls so the DVFS
# p-state ramps and real matmuls run at full clock.  Experimental.
_FILL_ENC = int(os.environ.get("KERNEL_FILL_ENC", "0"))
_FILL_DEC = int(os.environ.get("KERNEL_FILL_DEC", "0"))

_CACHE = {}


def _cvt(a):
    """Cast fp32 array to the matmul/state dtype grid."""
    a = np.ascontiguousarray(a, np.float32)
    if _MM_MODE == "f32":
        return a
    if _MM_MODE == "f16":
        return np.ascontiguousarray(a.astype(np.float16))
    if _MM_MODE == "bf16":
        import ml_dtypes
        return np.ascontiguousarray(a.astype(ml_dtypes.bfloat16))
    bits = a.view(np.uint32)
    out = ((bits.astype(np.uint64) + 0x800) & 0xFFFFF000).astype(np.uint32)
    return out.view(np.float32)


def _build_program(with_b1h):
    """Build + schedule the single-core Bass program (shared by all 8
    cores; per-core data differs only through the input tensors)."""
    nc = bacc.Bacc("TRN2", target_bir_lowering=False, debug=False)

    x_seq = nc.dram_tensor("x_seq", [T_ENC, BC], RD, kind="ExternalInput").ap()
    eps_bm = nc.dram_tensor("eps_bm", [GAMMA - 1, U, NQ], F32,
                            kind="ExternalInput").ap()
    r_w = nc.dram_tensor("r_w", [U, 3 * U], RD, kind="ExternalInput").ap()
    k_w = nc.dram_tensor("k_w", [1, 3 * U], RD, kind="ExternalInput").ap()
    k_col = nc.dram_tensor("k_col", [U, 3], F32, kind="ExternalInput").ap()
    wkp = nc.dram_tensor("wkp", [U, 2 * GAMMA], RD, kind="ExternalInput").ap()
    wb0b = nc.dram_tensor("wb0b", [U, GAMMA], F32, kind="ExternalInput").ap()
    cb1b = nc.dram_tensor("cb1b", [U, GAMMA], F32, kind="ExternalInput").ap()
    gb = nc.dram_tensor("gb", [U, 4], F32, kind="ExternalInput").ap()
    gzd = nc.dram_tensor("gzd", [U, GAMMA - 1], F32, kind="ExternalInput").ap()
    grd = nc.dram_tensor("grd", [U, GAMMA - 1], F32, kind="ExternalInput").ap()
    ghd = nc.dram_tensor("ghd", [U, GAMMA - 1], F32, kind="ExternalInput").ap()
    idt = nc.dram_tensor("idt", [U, U], RD, kind="ExternalInput").ap()
    out_bm = nc.dram_tensor("out_bm", [GAMMA, 2, U, NQ], F32,
                            kind="ExternalOutput").ap()

    with tile.TileContext(nc) as tc, ExitStack() as es:
        consts = es.enter_context(tc.tile_pool(name="consts", bufs=1))
        R = consts.tile([U, 3 * U], RD)
        K = consts.tile([1, 3 * U], RD)
        KC = consts.tile([U, 3], F32)
        WKP = consts.tile([U, 2 * GAMMA], RD)
        WB0B = consts.tile([U, GAMMA], F32)
        CB1B = consts.tile([U, GAMMA], F32)
        GB = consts.tile([U, 4], F32)
        GZD = consts.tile([U, GAMMA - 1], F32)
        GRD = consts.tile([U, GAMMA - 1], F32)
        GHD = consts.tile([U, GAMMA - 1], F32)
        IDT = consts.tile([U, U], RD)
        nc.sync.dma_start(IDT[:], idt[:])
        nc.sync.dma_start(R[:], r_w[:])
        nc.sync.dma_start(K[:], k_w[:])
        nc.sync.dma_start(KC[:], k_col[:])
        nc.sync.dma_start(WKP[:], wkp[:])
        nc.sync.dma_start(WB0B[:], wb0b[:])
        nc.sync.dma_start(CB1B[:], cb1b[:])
        nc.sync.dma_start(GB[:], gb[:])
        nc.sync.dma_start(GZD[:], gzd[:])
        nc.sync.dma_start(GRD[:], grd[:])
        nc.sync.dma_start(GHD[:], ghd[:])

        hpool = es.enter_context(tc.tile_pool(name="h", bufs=2 * NCH))
        gates = es.enter_context(tc.tile_pool(name="gates", bufs=3))
        samp = es.enter_context(tc.tile_pool(name="samp", bufs=2))
        stage = es.enter_context(tc.tile_pool(name="stage", bufs=5))
        ps_g = es.enter_context(tc.tile_pool(
            name="psg", bufs=int(os.environ.get("KERNEL_PS_BUFS", "6")),
            space="PSUM"))

        z3 = bass.ts(0, U)   # gate column ranges in R/K
        r3 = bass.ts(1, U)
        h3 = bass.ts(2, U)

        h = []
        for c in range(NCH):
            hc = hpool.tile([U, CW], RD, tag=f"h{c}", name="h0")
            nc.vector.memset(hc[:], 0.0)
            h.append(hc)

        def gru_mms_enc(c, xb):
            """Encoder matmuls for chunk c: rank-1 K@x accumulated with
            R@h for the z/r gates (the h-gate x-term rides the DVE stt
            since the reset gate only scales the recurrent part)."""
            hc = h[c]
            x_row = xb[0:1, bass.ts(c, CW)]
            pz = ps_g.tile([U, CW], F32, tag="ps", name="pz")
            pr = ps_g.tile([U, CW], F32, tag="ps", name="pr")
            ph = ps_g.tile([U, CW], F32, tag="ps", name="ph")
            nc.tensor.matmul(pz[:], K[:, z3], x_row, start=True, stop=False)
            nc.tensor.matmul(pz[:], R[:, z3], hc[:], start=False, stop=True)
            nc.tensor.matmul(pr[:], K[:, r3], x_row, start=True, stop=False)
            nc.tensor.matmul(pr[:], R[:, r3], hc[:], start=False, stop=True)
            nc.tensor.matmul(ph[:], R[:, h3], hc[:], start=True, stop=True)
            return pz, pr, ph

        def gru_rh_mms_dec(c):
            """Decoder R@h matmuls for chunk c (issued early: they only
            need the previous step's h, and keep the PE busy while the
            sample chain runs)."""
            hc = h[c]
            pz = ps_g.tile([U, CW], F32, tag="ps", name="pz")
            pr = ps_g.tile([U, CW], F32, tag="ps", name="pr")
            ph = ps_g.tile([U, CW], F32, tag="ps", name="ph")
            nc.tensor.matmul(pz[:], R[:, z3], hc[:], start=True, stop=False)
            nc.tensor.matmul(pr[:], R[:, r3], hc[:], start=True, stop=False)
            nc.tensor.matmul(ph[:], R[:, h3], hc[:], start=True, stop=True)
            return pz, pr, ph

        def gru_ky_mms(c, pz, pr, y):
            """Decoder K@y matmuls for chunk c: z/r accumulate into the
            R@h banks; the h-gate needs its own bank (the reset gate
            scales only the recurrent part)."""
            y_row = y[0:1, bass.ts(c, CW)]
            px = ps_g.tile([U, CW], F32, tag="ps", name="px")
            nc.tensor.matmul(pz[:], K[:, z3], y_row, start=False, stop=True)
            nc.tensor.matmul(pr[:], K[:, r3], y_row, start=False, stop=True)
            nc.tensor.matmul(px[:], K[:, h3], y_row, start=True, stop=True)
            return px

        def gru_tails(pss, bias_z, bias_r, bias_h, xb=None, pxs=None,
                      uxs=None, phss=None):
            """Gate nonlinearities + state update for ALL chunks, in
            cross-chunk phases so no chunk's sigmoids queue behind
            another chunk's tanh on the in-order scalar engine.
            r_ is emitted first (tt, the chain head, needs it);
            h2 = (h - u1*h) + u1*hh, with hz = u1*h on the idle GpSimd
            engine in the tanh shadow.  uxs: precomputed x*K_h tiles
            (encoder); phss: fp16 SBUF copies of the R_h@h PSUM (decoder,
            made during the y-DMA window) letting tt run at 2x."""
            u1s, r_s, uus, fs = [], [], [], []
            for c in range(NCH):
                pz, pr, ph = pss[c]
                r_ = gates.tile([U, CW], RD16, tag=f"r_{c}", name="r_")
                nc.scalar.activation(r_[:], pr[:], AF.Sigmoid, bias=bias_r,
                                     scale=1.0)
                u1 = gates.tile([U, CW], RD16, tag=f"u1_{c}", name="u1")
                nc.scalar.activation(u1[:], pz[:], AF.Sigmoid, bias=bias_z,
                                     scale=-1.0)
                u1s.append(u1)
                r_s.append(r_)
            for c in range(NCH):
                pz, pr, ph = pss[c]
                hc = h[c]
                hrec = phss[c] if phss is not None else ph
                if with_b1h:
                    hb = gates.tile([U, CW], F32, tag=f"hb_{c}", name="hb")
                    nc.vector.tensor_scalar(hb[:], ph[:], GB[:, 3:4], None,
                                            op0=ALU.add)
                    hrec = hb
                tt = gates.tile([U, CW], RD16, tag=f"t_{c}", name="tt")
                nc.vector.tensor_mul(tt[:], r_s[c][:], hrec[:])
                uu = gates.tile([U, CW], RD16, tag=f"u_{c}", name="uu")
                if uxs is not None:
                    nc.vector.tensor_add(uu[:], tt[:], uxs[c][:])
                elif xb is not None:
                    nc.vector.scalar_tensor_tensor(
                        uu[:], xb[:, bass.ts(c, CW)], KC[:, 2:3], tt[:],
                        op0=ALU.mult, op1=ALU.add)
                else:
                    nc.vector.tensor_add(uu[:], tt[:], pxs[c][:])
                uus.append(uu)
                hz = gates.tile([U, CW], RD16, tag=f"hz_{c}", name="hz")
                nc.gpsimd.tensor_mul(hz[:], u1s[c][:], hc[:])
                f = gates.tile([U, CW], RD16, tag=f"f_{c}", name="f")
                nc.vector.tensor_sub(f[:], hc[:], hz[:])
                fs.append(f)
            hhs = []
            for c in range(NCH):
                hh = gates.tile([U, CW], RD16, tag=f"hh_{c}", name="hh")
                nc.scalar.activation(hh[:], uus[c][:], AF.Tanh, bias=bias_h,
                                     scale=1.0)
                hhs.append(hh)
            for c in range(NCH):
                g = gates.tile([U, CW], RD16, tag=f"g_{c}", name="g")
                nc.vector.tensor_mul(g[:], u1s[c][:], hhs[c][:])
                h2 = hpool.tile([U, CW], RD, tag=f"h{c}", name="h2")
                nc.vector.tensor_add(h2[:], fs[c][:], g[:])
                h[c] = h2

        def dense_var(t):
            """Batch-major dense head for step t: per 128-batch slice,
            lhsT = h-slice (stationary), rhs = [W0|W1] columns -> PSUM
            [128, 2*NQ] with batch on partitions.  Then one micro
            sigmoid (resident table)."""
            ps4 = ps_g.tile([U, 2 * NQ + (448 if _FILL_DEC else 0)], F32,
                            tag="ps4", bufs=1, name="ps4")
            for c in range(NCH):
                for j in range(SL):
                    q = SL * c + j
                    nc.tensor.matmul(
                        ps4[:, 2 * q: 2 * q + 2],
                        h[c][:, U * j: U * j + U],
                        WKP[:, 2 * t: 2 * t + 2],
                        start=True, stop=True)
            g4 = samp.tile([U, NQ], F32, tag="g4", name="g4")
            # g = sigmoid(-(v + C + wb1)) = e^{-softplus(v+C+wb1)}
            nc.scalar.activation(g4[:], ps4[:, 1:2*NQ:2], AF.Sigmoid,
                                 bias=CB1B[:, t: t + 1], scale=-1.0)
            return ps4, g4

        def sample_and_out(t, ps4, g4, pss=None, last=False):
            """DVE fastlog softplus + reparameterized sample + outputs.
            ln(g) = (e-127)*ln2 + poly(mantissa) via bitcast tricks; the
            sample m = (1e-5 + 0.05*sp)*eps; y = m + loc_raw (wb0 rides
            the next step's gate biases).  One small DMA flips y[128,NQ]
            into the row form the K@y matmuls need."""
            bb = g4[:].bitcast(U32)
            e_ = samp.tile([U, NQ], U32, tag="e_", name="e_")
            nc.vector.tensor_scalar(e_[:], bb, 23, None,
                                    op0=ALU.logical_shift_right)
            f_ = samp.tile([U, NQ], U32, tag="f_", name="f_")
            nc.vector.tensor_scalar(f_[:], bb, 0x007FFFFF, 0x3F800000,
                                    op0=ALU.bitwise_and, op1=ALU.bitwise_or)
            ff = f_[:].bitcast(F32)
            p1 = samp.tile([U, NQ], F32, tag="p1", name="p1")
            nc.vector.tensor_scalar(p1[:], ff, _C3, _C2, op0=ALU.mult,
                                    op1=ALU.add)
            p2 = samp.tile([U, NQ], F32, tag="p2", name="p2")
            nc.vector.tensor_tensor(p2[:], p1[:], ff, op=ALU.mult)
            p3 = samp.tile([U, NQ], F32, tag="p3", name="p3")
            nc.vector.scalar_tensor_tensor(p3[:], p2[:], _C1, ff,
                                           op0=ALU.add, op1=ALU.mult)
            p4 = samp.tile([U, NQ], F32, tag="p4", name="p4")
            nc.vector.scalar_tensor_tensor(p4[:], e_[:], _LN2, p3[:],
                                           op0=ALU.mult, op1=ALU.add)
            # outputs (off the critical chain; host unpermutes batch-major)
            loc4 = samp.tile([U, NQ], F32, tag="loc4", name="loc4")
            nc.vector.tensor_scalar(loc4[:], ps4[:, 0:2*NQ:2], WB0B[:, t: t + 1],
                                    None, op0=ALU.add)
            nc.sync.dma_start(out_bm[t: t + 1, 0:1], loc4[:])
            sc4 = samp.tile([U, NQ], F32, tag="sc4", name="sc4")
            nc.vector.tensor_scalar(sc4[:], p4[:], -OP_SCALE, _SAMP_S1,
                                    op0=ALU.mult, op1=ALU.add)
            nc.sync.dma_start(out_bm[t: t + 1, 1:2], sc4[:])
            if last:
                return None
            ep4 = stage.tile([U, NQ], F32, tag="eps", name="ep4")
            nc.sync.dma_start(ep4[:], eps_bm[t: t + 1])
            m4 = samp.tile([U, NQ], F32, tag="m4", name="m4")
            nc.vector._custom_dve(
                AFFINE_MUL_REDUCE, out=m4[:], in0=p4[:], in1=ep4[:],
                s0=-OP_SCALE, s1=_SAMP_S1)
            y4 = samp.tile([U, NQ], RD, tag="y4", name="y4")
            nc.vector.tensor_add(y4[:], m4[:], ps4[:, 0:2*NQ:2])
            # y4 [128, NQ] -> per-chunk PE transpose -> [SL, 128] PSUM
            # (chunk c at partition base 32c), scalar-copy to SBUF, then
            # a contiguous DMA lands each chunk's y row; chunk 0's K@y
            # starts while chunk 1's DMA is still in flight.
            yt = ps_g.tile([32 * (NCH - 1) + SL, U], RD, tag="yt", bufs=1,
                           name="yt")
            y = samp.tile([1, BC], RD, tag="y", name="y")
            for c in range(NCH):
                p0 = 32 * c
                nc.tensor.matmul(yt[p0:p0 + SL, :],
                                 y4[:, SL * c: SL * c + SL], IDT[:],
                                 is_transpose=True, skip_group_check=True)
                ys = samp.tile([SL, U], RD, tag=f"ys{c}", name="ys")
                nc.scalar.copy(ys[:], yt[p0:p0 + SL, :])
                nc.sync.dma_start(y[0:1, bass.ts(c, CW)], ys[:])
            # scratch matmuls into the unused tail of the ps4 bank keep
            # the PE streaming while it waits for the y DMA
            for _ in range(_FILL_DEC):
                nc.tensor.matmul(ps4[:, 2 * NQ: 2 * NQ + 448], R[:, z3],
                                 h[0][:, 0:448], start=True, stop=True)
            # R_h@h PSUM -> fp16 SBUF on the scalar engine: it is idle
            # during the y DMA, and the copy lets tt run at the 2x DVE
            # rate in the gate chain
            phss = []
            for c in range(NCH):
                phs = gates.tile([U, CW], RD16, tag=f"phs_{c}", name="phs")
                nc.scalar.copy(phs[:], pss[c][2][:])
                phss.append(phs)
            return y, phss

        # ---- encoder: 48 GRU steps over the input sequence ----
        for t in range(T_ENC):
            xb = stage.tile([U, BC], RD, tag="xb", name="xb")
            nc.sync.dma_start(xb[:], x_seq[t: t + 1, :].partition_broadcast(U))
            # x*K_h for the h-gate on the idle GpSimd engine (x is
            # prefetched, so this runs well ahead of the chain)
            uxs = []
            for c in range(NCH):
                ux = gates.tile([U, CW], RD16, tag=f"ux_{c}", name="ux")
                nc.gpsimd.tensor_scalar(ux[:], xb[:, bass.ts(c, CW)],
                                        KC[:, 2:3], None, op0=ALU.mult)
                uxs.append(ux)
            ps = [gru_mms_enc(c, xb) for c in range(NCH)]
            if _FILL_ENC:
                pf = ps_g.tile([U, 448], F32, tag="ps4", bufs=1, name="pf")
                for _ in range(_FILL_ENC):
                    nc.tensor.matmul(pf[:], R[:, z3], h[0][:, 0:448],
                                     start=True, stop=True)
            gru_tails(ps, GB[:, 0:1], GB[:, 1:2], GB[:, 2:3], uxs=uxs)

        # ---- decoder: dense head + 27 sampled feedback GRU steps ----
        ps4, g4 = dense_var(0)
        for t in range(1, GAMMA):
            j = t - 1
            # R@h first: they only need the previous h and keep the PE
            # busy under the sample chain.
            ps = [gru_rh_mms_dec(c) for c in range(NCH)]
            y, phss = sample_and_out(j, ps4, g4, pss=ps)
            pxs = [gru_ky_mms(c, ps[c][0], ps[c][1], y) for c in range(NCH)]
            gru_tails(ps, GZD[:, j:j + 1], GRD[:, j:j + 1],
                      GHD[:, j:j + 1], pxs=pxs, phss=phss)
            ps4, g4 = dense_var(t)
        sample_and_out(GAMMA - 1, ps4, g4, last=True)

    nc.compile()
    return nc


def _host_prep(inputs, gru_kernel, gru_rec_kernel, gru_bias, dv_loc, dv_rho,
               dv_eps, samp_eps):
    """Host-side input preprocessing -> per-core input maps."""
    inputs = np.asarray(inputs, np.float32)
    B = inputs.shape[0]
    assert B == B_FULL, f"kernel compiled for B={B_FULL}, got {B}"
    xT = _cvt(inputs[:, :T_ENC, 0].T)                          # [48, B]
    epsT = np.ascontiguousarray(np.asarray(samp_eps, np.float32)[:, :, 0])  # [27, B]

    gru_kernel = np.asarray(gru_kernel, np.float32)
    gru_bias = np.asarray(gru_bias, np.float32)
    b0, b1 = gru_bias[0], gru_bias[1]
    kz = gru_kernel[0, 0:U]
    kr = gru_kernel[0, U:2 * U]
    kh = gru_kernel[0, 2 * U:3 * U]
    gb = np.zeros((U, 4), np.float32)
    gb[:, 0] = -(b0[0:U] + b1[0:U])
    gb[:, 1] = b0[U: 2 * U] + b1[U: 2 * U]
    gb[:, 2] = b0[2 * U: 3 * U]
    gb[:, 3] = b1[2 * U: 3 * U]

    dv_loc = np.asarray(dv_loc, np.float32)
    dv_rho = np.asarray(dv_rho, np.float32)
    dv_eps = np.asarray(dv_eps, np.float32)
    scale_q = np.float32(1e-5) + np.float32(Q_SCALE) * np.logaddexp(
        np.float32(C_SP) + dv_rho, np.float32(0.0), dtype=np.float32)
    w_all = dv_loc[None, :] + scale_q[None, :] * dv_eps        # [28, 258]
    wkp = np.ascontiguousarray(
        w_all[:, : 2 * U].reshape(GAMMA, U, 2).transpose(1, 0, 2)
        .reshape(U, 2 * GAMMA))
    wb0 = w_all[:, 2 * U]                                      # [28]
    cb1 = -(np.float32(C_SP) + w_all[:, 2 * U + 1])            # [28]
    wb0b = np.ascontiguousarray(
        np.broadcast_to(wb0[None, :], (U, GAMMA)), np.float32)
    cb1b = np.ascontiguousarray(
        np.broadcast_to(cb1[None, :], (U, GAMMA)), np.float32)

    # decoder per-step gate biases with K_g*wb0 folded in (gru step t
    # consumes the dense head of step j=t-1 -> columns indexed by j)
    wb0d = wb0[: GAMMA - 1]
    gzd = -((b0[0:U] + b1[0:U])[:, None] + np.outer(kz, wb0d))  # [U, 27]
    grd = (b0[U:2 * U] + b1[U:2 * U])[:, None] + np.outer(kr, wb0d)
    ghd = b0[2 * U:3 * U][:, None] + np.outer(kh, wb0d)

    shared = {
        "r_w": _cvt(gru_rec_kernel),
        "k_w": _cvt(gru_kernel),
        "k_col": np.ascontiguousarray(gru_kernel.reshape(3, U).T),
        "wkp": _cvt(wkp),
        "wb0b": wb0b,
        "cb1b": cb1b,
        "gb": gb,
        "gzd": np.ascontiguousarray(gzd, np.float32),
        "grd": np.ascontiguousarray(grd, np.float32),
        "ghd": np.ascontiguousarray(ghd, np.float32),
        "idt": _cvt(np.eye(U, dtype=np.float32)),
    }
    in_maps = []
    for c in range(N_CORES):
        sl = slice(c * BC, (c + 1) * BC)
        # eps batch-major: eps_bm[t, p, q] = eps[t, 512*(q//SL)+128*(q%SL)+p]
        e = epsT[:, sl].reshape(GAMMA - 1, NCH, SL, U)
        e = np.ascontiguousarray(e.transpose(0, 3, 1, 2).reshape(
            GAMMA - 1, U, NQ))
        in_maps.append(
            dict(
                shared,
                x_seq=np.ascontiguousarray(xT[:, sl]),
                eps_bm=e,
            )
        )
    return in_maps, bool(np.any(gb[:, 3] != 0.0))


def _get_nc(with_b1h=False):
    key = ("nc", with_b1h)
    if key not in _CACHE:
        _CACHE[key] = _build_program(with_b1h)
    return _CACHE[key]


def _unpack_out(fm):
    """[GAMMA, 2, U, NQ] batch-major -> [BC, GAMMA, 2]."""
    a = fm.transpose(2, 3, 0, 1)                      # [p, q, t, k]
    a = a.reshape(U, NCH, SL, GAMMA, 2)               # [p, c, j, t, k]
    return a.transpose(1, 2, 0, 3, 4).reshape(BC, GAMMA, 2)


def run(inputs_dict, trace=False, trace_kwargs=None):
    in_maps, with_b1h = _host_prep(**inputs_dict)
    nc = _get_nc(with_b1h)
    res = run_bass_kernel_spmd(
        nc, in_maps, list(range(N_CORES)), trace=trace,
        **(trace_kwargs or {}),
    )
    _CACHE["last_results"] = res
    out = np.empty((B_FULL, GAMMA, 2), np.float32)
    for c in range(N_CORES):
        out[c * BC: (c + 1) * BC] = _unpack_out(res.results[c]["out_bm"])
    return out


def kernel(**inputs):
    return run(inputs, trace=bool(os.environ.get("KERNEL_TRACE")))


# revision 49
# speedup vs baseline: 1.8666x; 1.8666x over previous
"""Trainium2 Bass kernel for nn_FIB_RNN (GRU encoder + autoregressive
sampling decoder with DenseVariational head).

Contract: kernel(**inputs) takes the FULL unsharded inputs (numpy arrays,
keys as in reference.setup_inputs()) and returns the FULL output
[B, GAMMA, 2] float32.

Strategy: pure data parallelism over the batch dim across 8 NeuronCores
(1024 batch rows per core).  The GRU state is feature-major
[U=128 partitions, batch free]; the recurrent matmul is
lhsT=R_gate[128,128] @ rhs=h[128,CW] -> PSUM, with the rank-1 K@x / K@y
input terms accumulated into the same banks.  fp16 end-to-end.

Decoder sample path (v3): the dense head runs BATCH-MAJOR -- for each
128-batch slice, lhsT = h-slice [U,128] (stationary), rhs = the step's
[W0|W1] columns [U,2], landing loc/v as [128, 2*NQ] PSUM with batch on
partitions.  The whole softplus/sample pipeline is then micro-ops:
sigmoid on [128,8] (the sigmoid table stays resident -- ZERO act-table
loads in the entire kernel), ln via exponent-extraction fastlog on the
vector engine (bitcast + shift/mask + deg-3 poly), reparameterized
sample, and one small DMA to flip y back to row form for the K@y
matmuls.  loc/scale outputs stream batch-major to DRAM and the host
unpermutes.
"""

import os
import sys
from contextlib import ExitStack

import numpy as np

for _p in ("/opt/trn_rl_repo", "/root/.axon_site/_ro/trn_rl_repo"):
    if os.path.isdir(_p) and _p not in sys.path:
        sys.path.insert(0, _p)

import concourse.bass as bass
import concourse.tile as tile
from concourse import bacc, mybir
from concourse.bass_utils import run_bass_kernel_spmd
from concourse.dve_ops import AFFINE_MUL_REDUCE

F32 = mybir.dt.float32
U32 = mybir.dt.uint32
I32 = mybir.dt.int32
AF = mybir.ActivationFunctionType
ALU = mybir.AluOpType

U = 128                    # rnn units
T_ENC = 48                 # encoder steps
GAMMA = 28                 # decoder outputs (27 sampled feedback steps)
N_CORES = 8
B_FULL = 8192
BC = B_FULL // N_CORES     # 1024 batch rows per core
CW = int(os.environ.get("KERNEL_CW", "512"))
NCH = BC // CW             # chunks per core
SL = CW // U               # 128-batch slices per chunk
NQ = BC // U               # slices per core (= NCH*SL)
C_SP = float(np.log(np.expm1(1.0)))  # softplus^-1(1.0)
Q_SCALE = 0.02
OP_SCALE = 0.05

# fastlog: ln(f) on [1,2) as c3 f^3 + c2 f^2 + c1 f + c0 (least squares,
# max err ~1e-4); ln(g) = (e_bits - 127)*ln2 + poly(mantissa)
_FS = np.linspace(1.0, 2.0, 8193)
_C3, _C2, _C1, _C0 = [float(v) for v in np.polyfit(_FS, np.log(_FS), 3)]
_LN2 = float(np.log(2.0))
_KLN = _C0 - 127.0 * _LN2          # ln(g) = p4 + _KLN given p4 below
_SAMP_S1 = 1e-5 - OP_SCALE * _KLN  # (1e-5 + 0.05*sp) = _SAMP_S1 - 0.05*p4

_MM_MODE = os.environ.get("KERNEL_MM_DT", "f16")
RD = {
    "f16": mybir.dt.float16,
    "f32r": mybir.dt.float32r,
    "bf16": mybir.dt.bfloat16,
    "f32": F32,
}[_MM_MODE]
RD16 = {
    "f16": mybir.dt.float16,
    "f32r": F32,
    "bf16": mybir.dt.bfloat16,
    "f32": F32,
}[_MM_MODE]

# PE filler matmuls per step (scratch PSUM, no readers): keep the
# tensor engine streaming through its dependency stalls so the DVFS
# p-state ramps and real matmuls run at full clock.  Experimental.
_FILL_ENC = int(os.environ.get("KERNEL_FILL_ENC", "0"))
_FILL_DEC = int(os.environ.get("KERNEL_FILL_DEC", "0"))

_CACHE = {}


def _cvt(a):
    """Cast fp32 array to the matmul/state dtype grid."""
    a = np.ascontiguousarray(a, np.float32)
    if _MM_MODE == "f32":
        return a
    if _MM_MODE == "f16":
        return np.ascontiguousarray(a.astype(np.float16))
    if _MM_MODE == "bf16":
        import ml_dtypes
        return np.ascontiguousarray(a.astype(ml_dtypes.bfloat16))
    bits = a.view(np.uint32)
    out = ((bits.astype(np.uint64) + 0x800) & 0xFFFFF000).astype(np.uint32)
    return out.view(np.float32)


def _build_program(with_b1h):
    """Build + schedule the single-core Bass program (shared by all 8
    cores; per-core data differs only through the input tensors)."""
    nc = bacc.Bacc("TRN2", target_bir_lowering=False, debug=False)

    x_seq = nc.dram_tensor("x_seq", [T_ENC, BC], RD, kind="ExternalInput").ap()
    eps_bm = nc.dram_tensor("eps_bm", [GAMMA - 1, U, NQ], F32,
                            kind="ExternalInput").ap()
    r_w = nc.dram_tensor("r_w", [U, 3 * U], RD, kind="ExternalInput").ap()
    k_w = nc.dram_tensor("k_w", [1, 3 * U], RD, kind="ExternalInput").ap()
    k_col = nc.dram_tensor("k_col", [U, 3], F32, kind="ExternalInput").ap()
    wkp = nc.dram_tensor("wkp", [U, 2 * GAMMA], RD, kind="ExternalInput").ap()
    wb0b = nc.dram_tensor("wb0b", [U, GAMMA], F32, kind="ExternalInput").ap()
    cb1b = nc.dram_tensor("cb1b", [U, GAMMA], F32, kind="ExternalInput").ap()
    gb = nc.dram_tensor("gb", [U, 4], F32, kind="ExternalInput").ap()
    gzd = nc.dram_tensor("gzd", [U, GAMMA - 1], F32, kind="ExternalInput").ap()
    grd = nc.dram_tensor("grd", [U, GAMMA - 1], F32, kind="ExternalInput").ap()
    ghd = nc.dram_tensor("ghd", [U, GAMMA - 1], F32, kind="ExternalInput").ap()
    idt = nc.dram_tensor("idt", [U, U], RD, kind="ExternalInput").ap()
    out_bm = nc.dram_tensor("out_bm", [GAMMA, 2, U, NQ], F32,
                            kind="ExternalOutput").ap()

    with tile.TileContext(nc) as tc, ExitStack() as es:
        consts = es.enter_context(tc.tile_pool(name="consts", bufs=1))
        R = consts.tile([U, 3 * U], RD)
        K = consts.tile([1, 3 * U], RD)
        KC = consts.tile([U, 3], F32)
        WKP = consts.tile([U, 2 * GAMMA], RD)
        WB0B = consts.tile([U, GAMMA], F32)
        CB1B = consts.tile([U, GAMMA], F32)
        GB = consts.tile([U, 4], F32)
        GZD = consts.tile([U, GAMMA - 1], F32)
        GRD = consts.tile([U, GAMMA - 1], F32)
        GHD = consts.tile([U, GAMMA - 1], F32)
        IDT = consts.tile([U, U], RD)
        nc.sync.dma_start(IDT[:], idt[:])
        nc.sync.dma_start(R[:], r_w[:])
        nc.sync.dma_start(K[:], k_w[:])
        nc.sync.dma_start(KC[:], k_col[:])
        nc.sync.dma_start(WKP[:], wkp[:])
        nc.sync.dma_start(WB0B[:], wb0b[:])
        nc.sync.dma_start(CB1B[:], cb1b[:])
        nc.sync.dma_start(GB[:], gb[:])
        nc.sync.dma_start(GZD[:], gzd[:])
        nc.sync.dma_start(GRD[:], grd[:])
        nc.sync.dma_start(GHD[:], ghd[:])

        hpool = es.enter_context(tc.tile_pool(name="h", bufs=2 * NCH))
        gates = es.enter_context(tc.tile_pool(name="gates", bufs=3))
        samp = es.enter_context(tc.tile_pool(name="samp", bufs=2))
        stage = es.enter_context(tc.tile_pool(name="stage", bufs=5))
        ps_g = es.enter_context(tc.tile_pool(
            name="psg", bufs=int(os.environ.get("KERNEL_PS_BUFS", "6")),
            space="PSUM"))

        z3 = bass.ts(0, U)   # gate column ranges in R/K
        r3 = bass.ts(1, U)
        h3 = bass.ts(2, U)

        h = []
        for c in range(NCH):
            hc = hpool.tile([U, CW], RD, tag=f"h{c}", name="h0")
            nc.vector.memset(hc[:], 0.0)
            h.append(hc)

        def gru_mms_enc(c, xb):
            """Encoder matmuls for chunk c: rank-1 K@x accumulated with
            R@h for the z/r gates (the h-gate x-term rides the DVE stt
            since the reset gate only scales the recurrent part)."""
            hc = h[c]
            x_row = xb[0:1, bass.ts(c, CW)]
            pz = ps_g.tile([U, CW], F32, tag="ps", name="pz")
            pr = ps_g.tile([U, CW], F32, tag="ps", name="pr")
            ph = ps_g.tile([U, CW], F32, tag="ps", name="ph")
            nc.tensor.matmul(pz[:], K[:, z3], x_row, start=True, stop=False)
            nc.tensor.matmul(pz[:], R[:, z3], hc[:], start=False, stop=True)
            nc.tensor.matmul(pr[:], K[:, r3], x_row, start=True, stop=False)
            nc.tensor.matmul(pr[:], R[:, r3], hc[:], start=False, stop=True)
            nc.tensor.matmul(ph[:], R[:, h3], hc[:], start=True, stop=True)
            return pz, pr, ph

        def gru_rh_mms_dec(c):
            """Decoder R@h matmuls for chunk c (issued early: they only
            need the previous step's h, and keep the PE busy while the
            sample chain runs)."""
            hc = h[c]
            pz = ps_g.tile([U, CW], F32, tag="ps", name="pz")
            pr = ps_g.tile([U, CW], F32, tag="ps", name="pr")
            ph = ps_g.tile([U, CW], F32, tag="ps", name="ph")
            nc.tensor.matmul(pz[:], R[:, z3], hc[:], start=True, stop=False)
            nc.tensor.matmul(pr[:], R[:, r3], hc[:], start=True, stop=False)
            nc.tensor.matmul(ph[:], R[:, h3], hc[:], start=True, stop=True)
            return pz, pr, ph

        def gru_ky_mms(c, pz, pr, y):
            """Decoder K@y matmuls for chunk c: z/r accumulate into the
            R@h banks; the h-gate needs its own bank (the reset gate
            scales only the recurrent part)."""
            y_row = y[0:1, bass.ts(c, CW)]
            px = ps_g.tile([U, CW], F32, tag="ps", name="px")
            nc.tensor.matmul(pz[:], K[:, z3], y_row, start=False, stop=True)
            nc.tensor.matmul(pr[:], K[:, r3], y_row, start=False, stop=True)
            nc.tensor.matmul(px[:], K[:, h3], y_row, start=True, stop=True)
            return px

        def gru_tails(pss, bias_z, bias_r, bias_h, xb=None, pxs=None,
                      uxs=None, phss=None):
            """Gate nonlinearities + state update for ALL chunks, in
            cross-chunk phases so no chunk's sigmoids queue behind
            another chunk's tanh on the in-order scalar engine.
            r_ is emitted first (tt, the chain head, needs it);
            h2 = (h - u1*h) + u1*hh, with hz = u1*h on the idle GpSimd
            engine in the tanh shadow.  uxs: precomputed x*K_h tiles
            (encoder); phss: fp16 SBUF copies of the R_h@h PSUM (decoder,
            made during the y-DMA window) letting tt run at 2x."""
            u1s, r_s, uus, fs = [], [], [], []
            for c in range(NCH):
                pz, pr, ph = pss[c]
                r_ = gates.tile([U, CW], RD16, tag=f"r_{c}", name="r_")
                nc.scalar.activation(r_[:], pr[:], AF.Sigmoid, bias=bias_r,
                                     scale=1.0)
                u1 = gates.tile([U, CW], RD16, tag=f"u1_{c}", name="u1")
                nc.scalar.activation(u1[:], pz[:], AF.Sigmoid, bias=bias_z,
                                     scale=-1.0)
                u1s.append(u1)
                r_s.append(r_)
            for c in range(NCH):
                pz, pr, ph = pss[c]
                hc = h[c]
                hrec = phss[c] if phss is not None else ph
                if with_b1h:
                    hb = gates.tile([U, CW], F32, tag=f"hb_{c}", name="hb")
                    nc.vector.tensor_scalar(hb[:], ph[:], GB[:, 3:4], None,
                                            op0=ALU.add)
                    hrec = hb
                tt = gates.tile([U, CW], RD16, tag=f"t_{c}", name="tt")
                nc.vector.tensor_mul(tt[:], r_s[c][:], hrec[:])
                uu = gates.tile([U, CW], RD16, tag=f"u_{c}", name="uu")
                if uxs is not None:
                    nc.vector.tensor_add(uu[:], tt[:], uxs[c][:])
                elif xb is not None:
                    nc.vector.scalar_tensor_tensor(
                        uu[:], xb[:, bass.ts(c, CW)], KC[:, 2:3], tt[:],
                        op0=ALU.mult, op1=ALU.add)
                else:
                    nc.vector.tensor_add(uu[:], tt[:], pxs[c][:])
                uus.append(uu)
                hz = gates.tile([U, CW], RD16, tag=f"hz_{c}", name="hz")
                nc.vector.tensor_mul(hz[:], u1s[c][:], hc[:])
                f = gates.tile([U, CW], RD16, tag=f"f_{c}", name="f")
                nc.vector.tensor_sub(f[:], hc[:], hz[:])
                fs.append(f)
            hhs = []
            for c in range(NCH):
                hh = gates.tile([U, CW], RD16, tag=f"hh_{c}", name="hh")
                nc.scalar.activation(hh[:], uus[c][:], AF.Tanh, bias=bias_h,
                                     scale=1.0)
                hhs.append(hh)
            for c in range(NCH):
                g = gates.tile([U, CW], RD16, tag=f"g_{c}", name="g")
                nc.vector.tensor_mul(g[:], u1s[c][:], hhs[c][:])
                h2 = hpool.tile([U, CW], RD, tag=f"h{c}", name="h2")
                nc.vector.tensor_add(h2[:], fs[c][:], g[:])
                h[c] = h2

        def dense_var(t):
            """Batch-major dense head for step t: per 128-batch slice,
            lhsT = h-slice (stationary), rhs = [W0|W1] columns -> PSUM
            [128, 2*NQ] with batch on partitions.  Then one micro
            sigmoid (resident table)."""
            ps4 = ps_g.tile([U, 2 * NQ + (448 if _FILL_DEC else 0)], F32,
                            tag="ps4", bufs=1, name="ps4")
            for c in range(NCH):
                for j in range(SL):
                    q = SL * c + j
                    nc.tensor.matmul(
                        ps4[:, 2 * q: 2 * q + 2],
                        h[c][:, U * j: U * j + U],
                        WKP[:, 2 * t: 2 * t + 2],
                        start=True, stop=True)
            g4 = samp.tile([U, NQ], F32, tag="g4", name="g4")
            # g = sigmoid(-(v + C + wb1)) = e^{-softplus(v+C+wb1)}
            nc.scalar.activation(g4[:], ps4[:, 1:2*NQ:2], AF.Sigmoid,
                                 bias=CB1B[:, t: t + 1], scale=-1.0)
            return ps4, g4

        def sample_and_out(t, ps4, g4, pss=None, last=False):
            """DVE fastlog softplus + reparameterized sample + outputs.
            ln(g) = (e-127)*ln2 + poly(mantissa) via bitcast tricks; the
            sample m = (1e-5 + 0.05*sp)*eps; y = m + loc_raw (wb0 rides
            the next step's gate biases).  One small DMA flips y[128,NQ]
            into the row form the K@y matmuls need."""
            bb = g4[:].bitcast(U32)
            e_ = samp.tile([U, NQ], U32, tag="e_", name="e_")
            nc.vector.tensor_scalar(e_[:], bb, 23, None,
                                    op0=ALU.logical_shift_right)
            f_ = samp.tile([U, NQ], U32, tag="f_", name="f_")
            nc.vector.tensor_scalar(f_[:], bb, 0x007FFFFF, 0x3F800000,
                                    op0=ALU.bitwise_and, op1=ALU.bitwise_or)
            ff = f_[:].bitcast(F32)
            p1 = samp.tile([U, NQ], F32, tag="p1", name="p1")
            nc.vector.tensor_scalar(p1[:], ff, _C3, _C2, op0=ALU.mult,
                                    op1=ALU.add)
            p2 = samp.tile([U, NQ], F32, tag="p2", name="p2")
            nc.vector.tensor_tensor(p2[:], p1[:], ff, op=ALU.mult)
            p3 = samp.tile([U, NQ], F32, tag="p3", name="p3")
            nc.vector.scalar_tensor_tensor(p3[:], p2[:], _C1, ff,
                                           op0=ALU.add, op1=ALU.mult)
            p4 = samp.tile([U, NQ], F32, tag="p4", name="p4")
            nc.vector.scalar_tensor_tensor(p4[:], e_[:], _LN2, p3[:],
                                           op0=ALU.mult, op1=ALU.add)
            # outputs (off the critical chain; host unpermutes batch-major)
            loc4 = samp.tile([U, NQ], F32, tag="loc4", name="loc4")
            nc.vector.tensor_scalar(loc4[:], ps4[:, 0:2*NQ:2], WB0B[:, t: t + 1],
                                    None, op0=ALU.add)
            nc.sync.dma_start(out_bm[t: t + 1, 0:1], loc4[:])
            sc4 = samp.tile([U, NQ], F32, tag="sc4", name="sc4")
            nc.vector.tensor_scalar(sc4[:], p4[:], -OP_SCALE, _SAMP_S1,
                                    op0=ALU.mult, op1=ALU.add)
            nc.sync.dma_start(out_bm[t: t + 1, 1:2], sc4[:])
            if last:
                return None
            ep4 = stage.tile([U, NQ], F32, tag="eps", name="ep4")
            nc.sync.dma_start(ep4[:], eps_bm[t: t + 1])
            m4 = samp.tile([U, NQ], F32, tag="m4", name="m4")
            nc.vector._custom_dve(
                AFFINE_MUL_REDUCE, out=m4[:], in0=p4[:], in1=ep4[:],
                s0=-OP_SCALE, s1=_SAMP_S1)
            y4 = samp.tile([U, NQ], RD, tag="y4", name="y4")
            nc.vector.tensor_add(y4[:], m4[:], ps4[:, 0:2*NQ:2])
            # y4 [128, NQ] -> per-chunk PE transpose -> [SL, 128] PSUM
            # (chunk c at partition base 32c), scalar-copy to SBUF, then
            # a contiguous DMA lands each chunk's y row; chunk 0's K@y
            # starts while chunk 1's DMA is still in flight.
            yt = ps_g.tile([32 * (NCH - 1) + SL, U], RD, tag="yt", bufs=1,
                           name="yt")
            y = samp.tile([1, BC], RD, tag="y", name="y")
            for c in range(NCH):
                p0 = 32 * c
                nc.tensor.matmul(yt[p0:p0 + SL, :],
                                 y4[:, SL * c: SL * c + SL], IDT[:],
                                 is_transpose=True, skip_group_check=True)
                ys = samp.tile([SL, U], RD, tag=f"ys{c}", name="ys")
                nc.scalar.copy(ys[:], yt[p0:p0 + SL, :])
                nc.sync.dma_start(y[0:1, bass.ts(c, CW)], ys[:])
            # scratch matmuls into the unused tail of the ps4 bank keep
            # the PE streaming while it waits for the y DMA
            for _ in range(_FILL_DEC):
                nc.tensor.matmul(ps4[:, 2 * NQ: 2 * NQ + 448], R[:, z3],
                                 h[0][:, 0:448], start=True, stop=True)
            # R_h@h PSUM -> fp16 SBUF on the scalar engine: it is idle
            # during the y DMA, and the copy lets tt run at the 2x DVE
            # rate in the gate chain
            phss = []
            for c in range(NCH):
                phs = gates.tile([U, CW], RD16, tag=f"phs_{c}", name="phs")
                nc.scalar.copy(phs[:], pss[c][2][:])
                phss.append(phs)
            return y, phss

        # ---- encoder: 48 GRU steps over the input sequence ----
        for t in range(T_ENC):
            xb = stage.tile([U, BC], RD, tag="xb", name="xb")
            nc.sync.dma_start(xb[:], x_seq[t: t + 1, :].partition_broadcast(U))
            ps = [gru_mms_enc(c, xb) for c in range(NCH)]
            if _FILL_ENC:
                pf = ps_g.tile([U, 448], F32, tag="ps4", bufs=1, name="pf")
                for _ in range(_FILL_ENC):
                    nc.tensor.matmul(pf[:], R[:, z3], h[0][:, 0:448],
                                     start=True, stop=True)
            gru_tails(ps, GB[:, 0:1], GB[:, 1:2], GB[:, 2:3], xb=xb)

        # ---- decoder: dense head + 27 sampled feedback GRU steps ----
        ps4, g4 = dense_var(0)
        for t in range(1, GAMMA):
            j = t - 1
            # R@h first: they only need the previous h and keep the PE
            # busy under the sample chain.
            ps = [gru_rh_mms_dec(c) for c in range(NCH)]
            y, phss = sample_and_out(j, ps4, g4, pss=ps)
            pxs = [gru_ky_mms(c, ps[c][0], ps[c][1], y) for c in range(NCH)]
            gru_tails(ps, GZD[:, j:j + 1], GRD[:, j:j + 1],
                      GHD[:, j:j + 1], pxs=pxs, phss=phss)
            ps4, g4 = dense_var(t)
        sample_and_out(GAMMA - 1, ps4, g4, last=True)

    nc.compile()
    return nc


def _host_prep(inputs, gru_kernel, gru_rec_kernel, gru_bias, dv_loc, dv_rho,
               dv_eps, samp_eps):
    """Host-side input preprocessing -> per-core input maps."""
    inputs = np.asarray(inputs, np.float32)
    B = inputs.shape[0]
    assert B == B_FULL, f"kernel compiled for B={B_FULL}, got {B}"
    xT = _cvt(inputs[:, :T_ENC, 0].T)                          # [48, B]
    epsT = np.ascontiguousarray(np.asarray(samp_eps, np.float32)[:, :, 0])  # [27, B]

    gru_kernel = np.asarray(gru_kernel, np.float32)
    gru_bias = np.asarray(gru_bias, np.float32)
    b0, b1 = gru_bias[0], gru_bias[1]
    kz = gru_kernel[0, 0:U]
    kr = gru_kernel[0, U:2 * U]
    kh = gru_kernel[0, 2 * U:3 * U]
    gb = np.zeros((U, 4), np.float32)
    gb[:, 0] = -(b0[0:U] + b1[0:U])
    gb[:, 1] = b0[U: 2 * U] + b1[U: 2 * U]
    gb[:, 2] = b0[2 * U: 3 * U]
    gb[:, 3] = b1[2 * U: 3 * U]

    dv_loc = np.asarray(dv_loc, np.float32)
    dv_rho = np.asarray(dv_rho, np.float32)
    dv_eps = np.asarray(dv_eps, np.float32)
    scale_q = np.float32(1e-5) + np.float32(Q_SCALE) * np.logaddexp(
        np.float32(C_SP) + dv_rho, np.float32(0.0), dtype=np.float32)
    w_all = dv_loc[None, :] + scale_q[None, :] * dv_eps        # [28, 258]
    wkp = np.ascontiguousarray(
        w_all[:, : 2 * U].reshape(GAMMA, U, 2).transpose(1, 0, 2)
        .reshape(U, 2 * GAMMA))
    wb0 = w_all[:, 2 * U]                                      # [28]
    cb1 = -(np.float32(C_SP) + w_all[:, 2 * U + 1])            # [28]
    wb0b = np.ascontiguousarray(
        np.broadcast_to(wb0[None, :], (U, GAMMA)), np.float32)
    cb1b = np.ascontiguousarray(
        np.broadcast_to(cb1[None, :], (U, GAMMA)), np.float32)

    # decoder per-step gate biases with K_g*wb0 folded in (gru step t
    # consumes the dense head of step j=t-1 -> columns indexed by j)
    wb0d = wb0[: GAMMA - 1]
    gzd = -((b0[0:U] + b1[0:U])[:, None] + np.outer(kz, wb0d))  # [U, 27]
    grd = (b0[U:2 * U] + b1[U:2 * U])[:, None] + np.outer(kr, wb0d)
    ghd = b0[2 * U:3 * U][:, None] + np.outer(kh, wb0d)

    shared = {
        "r_w": _cvt(gru_rec_kernel),
        "k_w": _cvt(gru_kernel),
        "k_col": np.ascontiguousarray(gru_kernel.reshape(3, U).T),
        "wkp": _cvt(wkp),
        "wb0b": wb0b,
        "cb1b": cb1b,
        "gb": gb,
        "gzd": np.ascontiguousarray(gzd, np.float32),
        "grd": np.ascontiguousarray(grd, np.float32),
        "ghd": np.ascontiguousarray(ghd, np.float32),
        "idt": _cvt(np.eye(U, dtype=np.float32)),
    }
    in_maps = []
    for c in range(N_CORES):
        sl = slice(c * BC, (c + 1) * BC)
        # eps batch-major: eps_bm[t, p, q] = eps[t, 512*(q//SL)+128*(q%SL)+p]
        e = epsT[:, sl].reshape(GAMMA - 1, NCH, SL, U)
        e = np.ascontiguousarray(e.transpose(0, 3, 1, 2).reshape(
            GAMMA - 1, U, NQ))
        in_maps.append(
            dict(
                shared,
                x_seq=np.ascontiguousarray(xT[:, sl]),
                eps_bm=e,
            )
        )
    return in_maps, bool(np.any(gb[:, 3] != 0.0))


def _get_nc(with_b1h=False):
    key = ("nc", with_b1h)
    if key not in _CACHE:
        _CACHE[key] = _build_program(with_b1h)
    return _CACHE[key]


def _unpack_out(fm):
    """[GAMMA, 2, U, NQ] batch-major -> [BC, GAMMA, 2]."""
    a = fm.transpose(2, 3, 0, 1)                      # [p, q, t, k]
    a = a.reshape(U, NCH, SL, GAMMA, 2)               # [p, c, j, t, k]
    return a.transpose(1, 2, 0, 3, 4).reshape(BC, GAMMA, 2)


def run(inputs_dict, trace=False, trace_kwargs=None):
    in_maps, with_b1h = _host_prep(**inputs_dict)
    nc = _get_nc(with_b1h)
    res = run_bass_kernel_spmd(
        nc, in_maps, list(range(N_CORES)), trace=trace,
        **(trace_kwargs or {}),
    )
    _CACHE["last_results"] = res
    out = np.empty((B_FULL, GAMMA, 2), np.float32)
    for c in range(N_CORES):
        out[c * BC: (c + 1) * BC] = _unpack_out(res.results[c]["out_bm"])
    return out


def kernel(**inputs):
    return run(inputs, trace=bool(os.environ.get("KERNEL_TRACE")))


# revision 53
# speedup vs baseline: 1.8690x; 1.0013x over previous
"""Trainium2 Bass kernel for nn_FIB_RNN (GRU encoder + autoregressive
sampling decoder with DenseVariational head).

Contract: kernel(**inputs) takes the FULL unsharded inputs (numpy arrays,
keys as in reference.setup_inputs()) and returns the FULL output
[B, GAMMA, 2] float32.

Strategy: pure data parallelism over the batch dim across 8 NeuronCores
(1024 batch rows per core).  The GRU state is feature-major
[U=128 partitions, batch free]; the recurrent matmul is
lhsT=R_gate[128,128] @ rhs=h[128,CW] -> PSUM, with the rank-1 K@x / K@y
input terms accumulated into the same banks.  fp16 end-to-end.

Decoder sample path (v3): the dense head runs BATCH-MAJOR -- for each
128-batch slice, lhsT = h-slice [U,128] (stationary), rhs = the step's
[W0|W1] columns [U,2], landing loc/v as [128, 2*NQ] PSUM with batch on
partitions.  The whole softplus/sample pipeline is then micro-ops:
sigmoid on [128,8] (the sigmoid table stays resident -- ZERO act-table
loads in the entire kernel), ln via exponent-extraction fastlog on the
vector engine (bitcast + shift/mask + deg-3 poly), reparameterized
sample, and one small DMA to flip y back to row form for the K@y
matmuls.  loc/scale outputs stream batch-major to DRAM and the host
unpermutes.
"""

import os
import sys
from contextlib import ExitStack

import numpy as np

for _p in ("/opt/trn_rl_repo", "/root/.axon_site/_ro/trn_rl_repo"):
    if os.path.isdir(_p) and _p not in sys.path:
        sys.path.insert(0, _p)

import concourse.bass as bass
import concourse.tile as tile
from concourse import bacc, mybir
from concourse.bass_utils import run_bass_kernel_spmd
from concourse.dve_ops import AFFINE_MUL_REDUCE

F32 = mybir.dt.float32
U32 = mybir.dt.uint32
I32 = mybir.dt.int32
AF = mybir.ActivationFunctionType
ALU = mybir.AluOpType

U = 128                    # rnn units
T_ENC = 48                 # encoder steps
GAMMA = 28                 # decoder outputs (27 sampled feedback steps)
N_CORES = 8
B_FULL = 8192
BC = B_FULL // N_CORES     # 1024 batch rows per core
CW = int(os.environ.get("KERNEL_CW", "512"))
NCH = BC // CW             # chunks per core
SL = CW // U               # 128-batch slices per chunk
NQ = BC // U               # slices per core (= NCH*SL)
C_SP = float(np.log(np.expm1(1.0)))  # softplus^-1(1.0)
Q_SCALE = 0.02
OP_SCALE = 0.05

# fastlog: ln(f) on [1,2) as c3 f^3 + c2 f^2 + c1 f + c0 (least squares,
# max err ~1e-4); ln(g) = (e_bits - 127)*ln2 + poly(mantissa)
_FS = np.linspace(1.0, 2.0, 8193)
_C3, _C2, _C1, _C0 = [float(v) for v in np.polyfit(_FS, np.log(_FS), 3)]
_LN2 = float(np.log(2.0))
_KLN = _C0 - 127.0 * _LN2          # ln(g) = p4 + _KLN given p4 below
_SAMP_S1 = 1e-5 - OP_SCALE * _KLN  # (1e-5 + 0.05*sp) = _SAMP_S1 - 0.05*p4

_MM_MODE = os.environ.get("KERNEL_MM_DT", "f16")
RD = {
    "f16": mybir.dt.float16,
    "f32r": mybir.dt.float32r,
    "bf16": mybir.dt.bfloat16,
    "f32": F32,
}[_MM_MODE]
RD16 = {
    "f16": mybir.dt.float16,
    "f32r": F32,
    "bf16": mybir.dt.bfloat16,
    "f32": F32,
}[_MM_MODE]

# PE filler matmuls per step (scratch PSUM, no readers): keep the
# tensor engine streaming through its dependency stalls so the DVFS
# p-state ramps and real matmuls run at full clock.  Experimental.
_FILL_ENC = int(os.environ.get("KERNEL_FILL_ENC", "0"))
_FILL_DEC = int(os.environ.get("KERNEL_FILL_DEC", "0"))

_CACHE = {}


def _cvt(a):
    """Cast fp32 array to the matmul/state dtype grid."""
    a = np.ascontiguousarray(a, np.float32)
    if _MM_MODE == "f32":
        return a
    if _MM_MODE == "f16":
        return np.ascontiguousarray(a.astype(np.float16))
    if _MM_MODE == "bf16":
        import ml_dtypes
        return np.ascontiguousarray(a.astype(ml_dtypes.bfloat16))
    bits = a.view(np.uint32)
    out = ((bits.astype(np.uint64) + 0x800) & 0xFFFFF000).astype(np.uint32)
    return out.view(np.float32)


def _build_program(with_b1h):
    """Build + schedule the single-core Bass program (shared by all 8
    cores; per-core data differs only through the input tensors)."""
    nc = bacc.Bacc("TRN2", target_bir_lowering=False, debug=False)

    x_seq = nc.dram_tensor("x_seq", [T_ENC, BC], RD, kind="ExternalInput").ap()
    eps_bm = nc.dram_tensor("eps_bm", [GAMMA - 1, U, NQ], F32,
                            kind="ExternalInput").ap()
    r_w = nc.dram_tensor("r_w", [U, 3 * U], RD, kind="ExternalInput").ap()
    k_w = nc.dram_tensor("k_w", [1, 3 * U], RD, kind="ExternalInput").ap()
    k_col = nc.dram_tensor("k_col", [U, 3], F32, kind="ExternalInput").ap()
    wkp = nc.dram_tensor("wkp", [U, 2 * GAMMA], RD, kind="ExternalInput").ap()
    wb0b = nc.dram_tensor("wb0b", [U, GAMMA], F32, kind="ExternalInput").ap()
    cb1b = nc.dram_tensor("cb1b", [U, GAMMA], F32, kind="ExternalInput").ap()
    gb = nc.dram_tensor("gb", [U, 4], F32, kind="ExternalInput").ap()
    gzd = nc.dram_tensor("gzd", [U, GAMMA - 1], F32, kind="ExternalInput").ap()
    grd = nc.dram_tensor("grd", [U, GAMMA - 1], F32, kind="ExternalInput").ap()
    ghd = nc.dram_tensor("ghd", [U, GAMMA - 1], F32, kind="ExternalInput").ap()
    idt = nc.dram_tensor("idt", [U, U], RD, kind="ExternalInput").ap()
    out_bm = nc.dram_tensor("out_bm", [GAMMA, 2, U, NQ], F32,
                            kind="ExternalOutput").ap()

    with tile.TileContext(nc) as tc, ExitStack() as es:
        consts = es.enter_context(tc.tile_pool(name="consts", bufs=1))
        R = consts.tile([U, 3 * U], RD)
        K = consts.tile([1, 3 * U], RD)
        KC = consts.tile([U, 3], F32)
        WKP = consts.tile([U, 2 * GAMMA], RD)
        WB0B = consts.tile([U, GAMMA], F32)
        CB1B = consts.tile([U, GAMMA], F32)
        GB = consts.tile([U, 4], F32)
        GZD = consts.tile([U, GAMMA - 1], F32)
        GRD = consts.tile([U, GAMMA - 1], F32)
        GHD = consts.tile([U, GAMMA - 1], F32)
        IDT = consts.tile([U, U], RD)
        nc.sync.dma_start(IDT[:], idt[:])
        nc.sync.dma_start(R[:], r_w[:])
        nc.sync.dma_start(K[:], k_w[:])
        nc.sync.dma_start(KC[:], k_col[:])
        nc.sync.dma_start(WKP[:], wkp[:])
        nc.sync.dma_start(WB0B[:], wb0b[:])
        nc.sync.dma_start(CB1B[:], cb1b[:])
        nc.sync.dma_start(GB[:], gb[:])
        nc.sync.dma_start(GZD[:], gzd[:])
        nc.sync.dma_start(GRD[:], grd[:])
        nc.sync.dma_start(GHD[:], ghd[:])

        hpool = es.enter_context(tc.tile_pool(name="h", bufs=2 * NCH))
        gates = es.enter_context(tc.tile_pool(name="gates", bufs=3))
        samp = es.enter_context(tc.tile_pool(name="samp", bufs=2))
        stage = es.enter_context(tc.tile_pool(name="stage", bufs=5))
        ps_g = es.enter_context(tc.tile_pool(
            name="psg", bufs=int(os.environ.get("KERNEL_PS_BUFS", "6")),
            space="PSUM"))

        z3 = bass.ts(0, U)   # gate column ranges in R/K
        r3 = bass.ts(1, U)
        h3 = bass.ts(2, U)

        h = []
        for c in range(NCH):
            hc = hpool.tile([U, CW], RD, tag=f"h{c}", name="h0")
            nc.vector.memset(hc[:], 0.0)
            h.append(hc)

        def gru_mms_enc(c, xb):
            """Encoder matmuls for chunk c: rank-1 K@x accumulated with
            R@h for the z/r gates (the h-gate x-term rides the DVE stt
            since the reset gate only scales the recurrent part)."""
            hc = h[c]
            x_row = xb[0:1, bass.ts(c, CW)]
            pz = ps_g.tile([U, CW], F32, tag="ps", name="pz")
            pr = ps_g.tile([U, CW], F32, tag="ps", name="pr")
            ph = ps_g.tile([U, CW], F32, tag="ps", name="ph")
            # r first (the chain head sigma_r waits on it), then the
            # h-gate (tt is next), z last (u1's consumers are in the
            # tanh shadow / post-tanh)
            nc.tensor.matmul(pr[:], K[:, r3], x_row, start=True, stop=False)
            nc.tensor.matmul(pr[:], R[:, r3], hc[:], start=False, stop=True)
            nc.tensor.matmul(ph[:], R[:, h3], hc[:], start=True, stop=True)
            nc.tensor.matmul(pz[:], K[:, z3], x_row, start=True, stop=False)
            nc.tensor.matmul(pz[:], R[:, z3], hc[:], start=False, stop=True)
            return pz, pr, ph

        def gru_rh_mms_dec(c):
            """Decoder R@h matmuls for chunk c (issued early: they only
            need the previous step's h, and keep the PE busy while the
            sample chain runs)."""
            hc = h[c]
            pz = ps_g.tile([U, CW], F32, tag="ps", name="pz")
            pr = ps_g.tile([U, CW], F32, tag="ps", name="pr")
            ph = ps_g.tile([U, CW], F32, tag="ps", name="ph")
            nc.tensor.matmul(pr[:], R[:, r3], hc[:], start=True, stop=False)
            nc.tensor.matmul(ph[:], R[:, h3], hc[:], start=True, stop=True)
            nc.tensor.matmul(pz[:], R[:, z3], hc[:], start=True, stop=False)
            return pz, pr, ph

        def gru_ky_mms(c, pz, pr, y):
            """Decoder K@y matmuls for chunk c: z/r accumulate into the
            R@h banks; the h-gate needs its own bank (the reset gate
            scales only the recurrent part)."""
            y_row = y[0:1, bass.ts(c, CW)]
            px = ps_g.tile([U, CW], F32, tag="ps", name="px")
            nc.tensor.matmul(pr[:], K[:, r3], y_row, start=False, stop=True)
            nc.tensor.matmul(px[:], K[:, h3], y_row, start=True, stop=True)
            nc.tensor.matmul(pz[:], K[:, z3], y_row, start=False, stop=True)
            return px

        def gru_tails(pss, bias_z, bias_r, bias_h, xb=None, pxs=None,
                      uxs=None, phss=None):
            """Gate nonlinearities + state update for ALL chunks, in
            cross-chunk phases so no chunk's sigmoids queue behind
            another chunk's tanh on the in-order scalar engine.
            r_ is emitted first (tt, the chain head, needs it);
            h2 = (h - u1*h) + u1*hh, with hz = u1*h on the idle GpSimd
            engine in the tanh shadow.  uxs: precomputed x*K_h tiles
            (encoder); phss: fp16 SBUF copies of the R_h@h PSUM (decoder,
            made during the y-DMA window) letting tt run at 2x."""
            u1s, r_s, uus, fs = [], [], [], []
            for c in range(NCH):
                pz, pr, ph = pss[c]
                r_ = gates.tile([U, CW], RD16, tag=f"r_{c}", name="r_")
                nc.scalar.activation(r_[:], pr[:], AF.Sigmoid, bias=bias_r,
                                     scale=1.0)
                u1 = gates.tile([U, CW], RD16, tag=f"u1_{c}", name="u1")
                nc.scalar.activation(u1[:], pz[:], AF.Sigmoid, bias=bias_z,
                                     scale=-1.0)
                u1s.append(u1)
                r_s.append(r_)
            for c in range(NCH):
                pz, pr, ph = pss[c]
                hc = h[c]
                hrec = phss[c] if phss is not None else ph
                if with_b1h:
                    hb = gates.tile([U, CW], F32, tag=f"hb_{c}", name="hb")
                    nc.vector.tensor_scalar(hb[:], ph[:], GB[:, 3:4], None,
                                            op0=ALU.add)
                    hrec = hb
                tt = gates.tile([U, CW], RD16, tag=f"t_{c}", name="tt")
                nc.vector.tensor_mul(tt[:], r_s[c][:], hrec[:])
                uu = gates.tile([U, CW], RD16, tag=f"u_{c}", name="uu")
                if uxs is not None:
                    nc.vector.tensor_add(uu[:], tt[:], uxs[c][:])
                elif xb is not None:
                    nc.vector.scalar_tensor_tensor(
                        uu[:], xb[:, bass.ts(c, CW)], KC[:, 2:3], tt[:],
                        op0=ALU.mult, op1=ALU.add)
                else:
                    nc.vector.tensor_add(uu[:], tt[:], pxs[c][:])
                uus.append(uu)
                hz = gates.tile([U, CW], RD16, tag=f"hz_{c}", name="hz")
                nc.vector.tensor_mul(hz[:], u1s[c][:], hc[:])
                f = gates.tile([U, CW], RD16, tag=f"f_{c}", name="f")
                nc.vector.tensor_sub(f[:], hc[:], hz[:])
                fs.append(f)
            hhs = []
            for c in range(NCH):
                hh = gates.tile([U, CW], RD16, tag=f"hh_{c}", name="hh")
                nc.scalar.activation(hh[:], uus[c][:], AF.Tanh, bias=bias_h,
                                     scale=1.0)
                hhs.append(hh)
            for c in range(NCH):
                g = gates.tile([U, CW], RD16, tag=f"g_{c}", name="g")
                nc.vector.tensor_mul(g[:], u1s[c][:], hhs[c][:])
                h2 = hpool.tile([U, CW], RD, tag=f"h{c}", name="h2")
                nc.vector.tensor_add(h2[:], fs[c][:], g[:])
                h[c] = h2

        def dense_var(t):
            """Batch-major dense head for step t: per 128-batch slice,
            lhsT = h-slice (stationary), rhs = [W0|W1] columns -> PSUM
            [128, 2*NQ] with batch on partitions.  Then one micro
            sigmoid (resident table)."""
            ps4 = ps_g.tile([U, 2 * NQ + (448 if _FILL_DEC else 0)], F32,
                            tag="ps4", bufs=1, name="ps4")
            for c in range(NCH):
                for j in range(SL):
                    q = SL * c + j
                    nc.tensor.matmul(
                        ps4[:, 2 * q: 2 * q + 2],
                        h[c][:, U * j: U * j + U],
                        WKP[:, 2 * t: 2 * t + 2],
                        start=True, stop=True)
            g4 = samp.tile([U, NQ], F32, tag="g4", name="g4")
            # g = sigmoid(-(v + C + wb1)) = e^{-softplus(v+C+wb1)}
            nc.scalar.activation(g4[:], ps4[:, 1:2*NQ:2], AF.Sigmoid,
                                 bias=CB1B[:, t: t + 1], scale=-1.0)
            return ps4, g4

        def sample_and_out(t, ps4, g4, pss=None, last=False):
            """DVE fastlog softplus + reparameterized sample + outputs.
            ln(g) = (e-127)*ln2 + poly(mantissa) via bitcast tricks; the
            sample m = (1e-5 + 0.05*sp)*eps; y = m + loc_raw (wb0 rides
            the next step's gate biases).  One small DMA flips y[128,NQ]
            into the row form the K@y matmuls need."""
            bb = g4[:].bitcast(U32)
            e_ = samp.tile([U, NQ], U32, tag="e_", name="e_")
            nc.vector.tensor_scalar(e_[:], bb, 23, None,
                                    op0=ALU.logical_shift_right)
            f_ = samp.tile([U, NQ], U32, tag="f_", name="f_")
            nc.vector.tensor_scalar(f_[:], bb, 0x007FFFFF, 0x3F800000,
                                    op0=ALU.bitwise_and, op1=ALU.bitwise_or)
            ff = f_[:].bitcast(F32)
            p1 = samp.tile([U, NQ], F32, tag="p1", name="p1")
            nc.vector.tensor_scalar(p1[:], ff, _C3, _C2, op0=ALU.mult,
                                    op1=ALU.add)
            p2 = samp.tile([U, NQ], F32, tag="p2", name="p2")
            nc.vector.tensor_tensor(p2[:], p1[:], ff, op=ALU.mult)
            p3 = samp.tile([U, NQ], F32, tag="p3", name="p3")
            nc.vector.scalar_tensor_tensor(p3[:], p2[:], _C1, ff,
                                           op0=ALU.add, op1=ALU.mult)
            p4 = samp.tile([U, NQ], F32, tag="p4", name="p4")
            nc.vector.scalar_tensor_tensor(p4[:], e_[:], _LN2, p3[:],
                                           op0=ALU.mult, op1=ALU.add)
            # outputs (off the critical chain; host unpermutes batch-major)
            loc4 = samp.tile([U, NQ], F32, tag="loc4", name="loc4")
            nc.vector.tensor_scalar(loc4[:], ps4[:, 0:2*NQ:2], WB0B[:, t: t + 1],
                                    None, op0=ALU.add)
            nc.sync.dma_start(out_bm[t: t + 1, 0:1], loc4[:])
            sc4 = samp.tile([U, NQ], F32, tag="sc4", name="sc4")
            nc.vector.tensor_scalar(sc4[:], p4[:], -OP_SCALE, _SAMP_S1,
                                    op0=ALU.mult, op1=ALU.add)
            nc.sync.dma_start(out_bm[t: t + 1, 1:2], sc4[:])
            if last:
                return None
            ep4 = stage.tile([U, NQ], F32, tag="eps", name="ep4")
            nc.sync.dma_start(ep4[:], eps_bm[t: t + 1])
            m4 = samp.tile([U, NQ], F32, tag="m4", name="m4")
            nc.vector._custom_dve(
                AFFINE_MUL_REDUCE, out=m4[:], in0=p4[:], in1=ep4[:],
                s0=-OP_SCALE, s1=_SAMP_S1)
            y4 = samp.tile([U, NQ], RD, tag="y4", name="y4")
            nc.vector.tensor_add(y4[:], m4[:], ps4[:, 0:2*NQ:2])
            # y4 [128, NQ] -> per-chunk PE transpose -> [SL, 128] PSUM
            # (chunk c at partition base 32c), scalar-copy to SBUF, then
            # a contiguous DMA lands each chunk's y row; chunk 0's K@y
            # starts while chunk 1's DMA is still in flight.
            yt = ps_g.tile([32 * (NCH - 1) + SL, U], RD, tag="yt", bufs=1,
                           name="yt")
            y = samp.tile([1, BC], RD, tag="y", name="y")
            for c in range(NCH):
                p0 = 32 * c
                nc.tensor.matmul(yt[p0:p0 + SL, :],
                                 y4[:, SL * c: SL * c + SL], IDT[:],
                                 is_transpose=True, skip_group_check=True)
                ys = samp.tile([SL, U], RD, tag=f"ys{c}", name="ys")
                nc.scalar.copy(ys[:], yt[p0:p0 + SL, :])
                nc.sync.dma_start(y[0:1, bass.ts(c, CW)], ys[:])
            # scratch matmuls into the unused tail of the ps4 bank keep
            # the PE streaming while it waits for the y DMA
            for _ in range(_FILL_DEC):
                nc.tensor.matmul(ps4[:, 2 * NQ: 2 * NQ + 448], R[:, z3],
                                 h[0][:, 0:448], start=True, stop=True)
            return y, None

        # ---- encoder: 48 GRU steps over the input sequence ----
        for t in range(T_ENC):
            xb = stage.tile([U, BC], RD, tag="xb", name="xb")
            nc.sync.dma_start(xb[:], x_seq[t: t + 1, :].partition_broadcast(U))
            ps = [gru_mms_enc(c, xb) for c in range(NCH)]
            if _FILL_ENC:
                pf = ps_g.tile([U, 448], F32, tag="ps4", bufs=1, name="pf")
                for _ in range(_FILL_ENC):
                    nc.tensor.matmul(pf[:], R[:, z3], h[0][:, 0:448],
                                     start=True, stop=True)
            gru_tails(ps, GB[:, 0:1], GB[:, 1:2], GB[:, 2:3], xb=xb)

        # ---- decoder: dense head + 27 sampled feedback GRU steps ----
        ps4, g4 = dense_var(0)
        for t in range(1, GAMMA):
            j = t - 1
            # R@h first: they only need the previous h and keep the PE
            # busy under the sample chain.
            ps = [gru_rh_mms_dec(c) for c in range(NCH)]
            y, phss = sample_and_out(j, ps4, g4, pss=ps)
            pxs = [gru_ky_mms(c, ps[c][0], ps[c][1], y) for c in range(NCH)]
            gru_tails(ps, GZD[:, j:j + 1], GRD[:, j:j + 1],
                      GHD[:, j:j + 1], pxs=pxs, phss=phss)
            ps4, g4 = dense_var(t)
        sample_and_out(GAMMA - 1, ps4, g4, last=True)

    nc.compile()
    return nc


def _host_prep(inputs, gru_kernel, gru_rec_kernel, gru_bias, dv_loc, dv_rho,
               dv_eps, samp_eps):
    """Host-side input preprocessing -> per-core input maps."""
    inputs = np.asarray(inputs, np.float32)
    B = inputs.shape[0]
    assert B == B_FULL, f"kernel compiled for B={B_FULL}, got {B}"
    xT = _cvt(inputs[:, :T_ENC, 0].T)                          # [48, B]
    epsT = np.ascontiguousarray(np.asarray(samp_eps, np.float32)[:, :, 0])  # [27, B]

    gru_kernel = np.asarray(gru_kernel, np.float32)
    gru_bias = np.asarray(gru_bias, np.float32)
    b0, b1 = gru_bias[0], gru_bias[1]
    kz = gru_kernel[0, 0:U]
    kr = gru_kernel[0, U:2 * U]
    kh = gru_kernel[0, 2 * U:3 * U]
    gb = np.zeros((U, 4), np.float32)
    gb[:, 0] = -(b0[0:U] + b1[0:U])
    gb[:, 1] = b0[U: 2 * U] + b1[U: 2 * U]
    gb[:, 2] = b0[2 * U: 3 * U]
    gb[:, 3] = b1[2 * U: 3 * U]

    dv_loc = np.asarray(dv_loc, np.float32)
    dv_rho = np.asarray(dv_rho, np.float32)
    dv_eps = np.asarray(dv_eps, np.float32)
    scale_q = np.float32(1e-5) + np.float32(Q_SCALE) * np.logaddexp(
        np.float32(C_SP) + dv_rho, np.float32(0.0), dtype=np.float32)
    w_all = dv_loc[None, :] + scale_q[None, :] * dv_eps        # [28, 258]
    wkp = np.ascontiguousarray(
        w_all[:, : 2 * U].reshape(GAMMA, U, 2).transpose(1, 0, 2)
        .reshape(U, 2 * GAMMA))
    wb0 = w_all[:, 2 * U]                                      # [28]
    cb1 = -(np.float32(C_SP) + w_all[:, 2 * U + 1])            # [28]
    wb0b = np.ascontiguousarray(
        np.broadcast_to(wb0[None, :], (U, GAMMA)), np.float32)
    cb1b = np.ascontiguousarray(
        np.broadcast_to(cb1[None, :], (U, GAMMA)), np.float32)

    # decoder per-step gate biases with K_g*wb0 folded in (gru step t
    # consumes the dense head of step j=t-1 -> columns indexed by j)
    wb0d = wb0[: GAMMA - 1]
    gzd = -((b0[0:U] + b1[0:U])[:, None] + np.outer(kz, wb0d))  # [U, 27]
    grd = (b0[U:2 * U] + b1[U:2 * U])[:, None] + np.outer(kr, wb0d)
    ghd = b0[2 * U:3 * U][:, None] + np.outer(kh, wb0d)

    shared = {
        "r_w": _cvt(gru_rec_kernel),
        "k_w": _cvt(gru_kernel),
        "k_col": np.ascontiguousarray(gru_kernel.reshape(3, U).T),
        "wkp": _cvt(wkp),
        "wb0b": wb0b,
        "cb1b": cb1b,
        "gb": gb,
        "gzd": np.ascontiguousarray(gzd, np.float32),
        "grd": np.ascontiguousarray(grd, np.float32),
        "ghd": np.ascontiguousarray(ghd, np.float32),
        "idt": _cvt(np.eye(U, dtype=np.float32)),
    }
    in_maps = []
    for c in range(N_CORES):
        sl = slice(c * BC, (c + 1) * BC)
        # eps batch-major: eps_bm[t, p, q] = eps[t, 512*(q//SL)+128*(q%SL)+p]
        e = epsT[:, sl].reshape(GAMMA - 1, NCH, SL, U)
        e = np.ascontiguousarray(e.transpose(0, 3, 1, 2).reshape(
            GAMMA - 1, U, NQ))
        in_maps.append(
            dict(
                shared,
                x_seq=np.ascontiguousarray(xT[:, sl]),
                eps_bm=e,
            )
        )
    return in_maps, bool(np.any(gb[:, 3] != 0.0))


def _get_nc(with_b1h=False):
    key = ("nc", with_b1h)
    if key not in _CACHE:
        _CACHE[key] = _build_program(with_b1h)
    return _CACHE[key]


def _unpack_out(fm):
    """[GAMMA, 2, U, NQ] batch-major -> [BC, GAMMA, 2]."""
    a = fm.transpose(2, 3, 0, 1)                      # [p, q, t, k]
    a = a.reshape(U, NCH, SL, GAMMA, 2)               # [p, c, j, t, k]
    return a.transpose(1, 2, 0, 3, 4).reshape(BC, GAMMA, 2)


def run(inputs_dict, trace=False, trace_kwargs=None):
    in_maps, with_b1h = _host_prep(**inputs_dict)
    nc = _get_nc(with_b1h)
    res = run_bass_kernel_spmd(
        nc, in_maps, list(range(N_CORES)), trace=trace,
        **(trace_kwargs or {}),
    )
    _CACHE["last_results"] = res
    out = np.empty((B_FULL, GAMMA, 2), np.float32)
    for c in range(N_CORES):
        out[c * BC: (c + 1) * BC] = _unpack_out(res.results[c]["out_bm"])
    return out


def kernel(**inputs):
    return run(inputs, trace=bool(os.environ.get("KERNEL_TRACE")))


# revision 55
# speedup vs baseline: 1.9557x; 1.0464x over previous
"""Trainium2 Bass kernel for nn_FIB_RNN (GRU encoder + autoregressive
sampling decoder with DenseVariational head).

Contract: kernel(**inputs) takes the FULL unsharded inputs (numpy arrays,
keys as in reference.setup_inputs()) and returns the FULL output
[B, GAMMA, 2] float32.

Strategy: pure data parallelism over the batch dim across 8 NeuronCores
(1024 batch rows per core).  The GRU state is feature-major
[U=128 partitions, batch free]; the recurrent matmul is
lhsT=R_gate[128,128] @ rhs=h[128,CW] -> PSUM, with the rank-1 K@x / K@y
input terms accumulated into the same banks.  fp16 end-to-end.

Decoder sample path (v3): the dense head runs BATCH-MAJOR -- for each
128-batch slice, lhsT = h-slice [U,128] (stationary), rhs = the step's
[W0|W1] columns [U,2], landing loc/v as [128, 2*NQ] PSUM with batch on
partitions.  The whole softplus/sample pipeline is then micro-ops:
sigmoid on [128,8] (the sigmoid table stays resident -- ZERO act-table
loads in the entire kernel), ln via exponent-extraction fastlog on the
vector engine (bitcast + shift/mask + deg-3 poly), reparameterized
sample, and one small DMA to flip y back to row form for the K@y
matmuls.  loc/scale outputs stream batch-major to DRAM and the host
unpermutes.
"""

import os
import sys
from contextlib import ExitStack

import numpy as np

for _p in ("/opt/trn_rl_repo", "/root/.axon_site/_ro/trn_rl_repo"):
    if os.path.isdir(_p) and _p not in sys.path:
        sys.path.insert(0, _p)

import concourse.bass as bass
import concourse.tile as tile
from concourse import bacc, mybir
from concourse.bass_utils import run_bass_kernel_spmd
from concourse.dve_ops import AFFINE_MUL_REDUCE

F32 = mybir.dt.float32
U32 = mybir.dt.uint32
I32 = mybir.dt.int32
AF = mybir.ActivationFunctionType
ALU = mybir.AluOpType

U = 128                    # rnn units
T_ENC = 48                 # encoder steps
GAMMA = 28                 # decoder outputs (27 sampled feedback steps)
N_CORES = 8
B_FULL = 8192
BC = B_FULL // N_CORES     # 1024 batch rows per core
CW = int(os.environ.get("KERNEL_CW", "512"))
NCH = BC // CW             # chunks per core
SL = CW // U               # 128-batch slices per chunk
NQ = BC // U               # slices per core (= NCH*SL)
C_SP = float(np.log(np.expm1(1.0)))  # softplus^-1(1.0)
Q_SCALE = 0.02
OP_SCALE = 0.05

# fastlog: ln(f) on [1,2) as c3 f^3 + c2 f^2 + c1 f + c0 (least squares,
# max err ~1e-4); ln(g) = (e_bits - 127)*ln2 + poly(mantissa)
_FS = np.linspace(1.0, 2.0, 8193)
_C3, _C2, _C1, _C0 = [float(v) for v in np.polyfit(_FS, np.log(_FS), 3)]
_LN2 = float(np.log(2.0))
_KLN = _C0 - 127.0 * _LN2          # ln(g) = p4 + _KLN given p4 below
_SAMP_S1 = 1e-5 - OP_SCALE * _KLN  # (1e-5 + 0.05*sp) = _SAMP_S1 - 0.05*p4

_MM_MODE = os.environ.get("KERNEL_MM_DT", "f16")
RD = {
    "f16": mybir.dt.float16,
    "f32r": mybir.dt.float32r,
    "bf16": mybir.dt.bfloat16,
    "f32": F32,
}[_MM_MODE]
RD16 = {
    "f16": mybir.dt.float16,
    "f32r": F32,
    "bf16": mybir.dt.bfloat16,
    "f32": F32,
}[_MM_MODE]

# PE filler matmuls per step (scratch PSUM, no readers): keep the
# tensor engine streaming through its dependency stalls so the DVFS
# p-state ramps and real matmuls run at full clock.  Experimental.
_FILL_ENC = int(os.environ.get("KERNEL_FILL_ENC", "0"))
_FILL_DEC = int(os.environ.get("KERNEL_FILL_DEC", "0"))

_CACHE = {}


def _cvt(a):
    """Cast fp32 array to the matmul/state dtype grid."""
    a = np.ascontiguousarray(a, np.float32)
    if _MM_MODE == "f32":
        return a
    if _MM_MODE == "f16":
        return np.ascontiguousarray(a.astype(np.float16))
    if _MM_MODE == "bf16":
        import ml_dtypes
        return np.ascontiguousarray(a.astype(ml_dtypes.bfloat16))
    bits = a.view(np.uint32)
    out = ((bits.astype(np.uint64) + 0x800) & 0xFFFFF000).astype(np.uint32)
    return out.view(np.float32)


def _build_program(with_b1h):
    """Build + schedule the single-core Bass program (shared by all 8
    cores; per-core data differs only through the input tensors)."""
    nc = bacc.Bacc("TRN2", target_bir_lowering=False, debug=False)

    x_seq = nc.dram_tensor("x_seq", [T_ENC, BC], RD, kind="ExternalInput").ap()
    eps_bm = nc.dram_tensor("eps_bm", [GAMMA - 1, U, NQ], F32,
                            kind="ExternalInput").ap()
    r_w = nc.dram_tensor("r_w", [U, 3 * U], RD, kind="ExternalInput").ap()
    k_w = nc.dram_tensor("k_w", [1, 3 * U], RD, kind="ExternalInput").ap()
    k_col = nc.dram_tensor("k_col", [U, 3], F32, kind="ExternalInput").ap()
    wkp = nc.dram_tensor("wkp", [U, 2 * GAMMA], RD, kind="ExternalInput").ap()
    wb0b = nc.dram_tensor("wb0b", [U, GAMMA], F32, kind="ExternalInput").ap()
    cb1b = nc.dram_tensor("cb1b", [U, GAMMA], F32, kind="ExternalInput").ap()
    gb = nc.dram_tensor("gb", [U, 4], F32, kind="ExternalInput").ap()
    gzd = nc.dram_tensor("gzd", [U, GAMMA - 1], F32, kind="ExternalInput").ap()
    grd = nc.dram_tensor("grd", [U, GAMMA - 1], F32, kind="ExternalInput").ap()
    ghd = nc.dram_tensor("ghd", [U, GAMMA - 1], F32, kind="ExternalInput").ap()
    idt = nc.dram_tensor("idt", [U, U], RD, kind="ExternalInput").ap()
    out_bm = nc.dram_tensor("out_bm", [GAMMA, 2, U, NQ], F32,
                            kind="ExternalOutput").ap()

    with tile.TileContext(nc) as tc, ExitStack() as es:
        consts = es.enter_context(tc.tile_pool(name="consts", bufs=1))
        R = consts.tile([U, 3 * U], RD)
        K = consts.tile([1, 3 * U], RD)
        KC = consts.tile([U, 3], F32)
        WKP = consts.tile([U, 2 * GAMMA], RD)
        WB0B = consts.tile([U, GAMMA], F32)
        CB1B = consts.tile([U, GAMMA], F32)
        GB = consts.tile([U, 4], F32)
        GZD = consts.tile([U, GAMMA - 1], F32)
        GRD = consts.tile([U, GAMMA - 1], F32)
        GHD = consts.tile([U, GAMMA - 1], F32)
        IDT = consts.tile([U, U], RD)
        nc.sync.dma_start(IDT[:], idt[:])
        nc.sync.dma_start(R[:], r_w[:])
        nc.sync.dma_start(K[:], k_w[:])
        nc.sync.dma_start(KC[:], k_col[:])
        nc.sync.dma_start(WKP[:], wkp[:])
        nc.sync.dma_start(WB0B[:], wb0b[:])
        nc.sync.dma_start(CB1B[:], cb1b[:])
        nc.sync.dma_start(GB[:], gb[:])
        nc.sync.dma_start(GZD[:], gzd[:])
        nc.sync.dma_start(GRD[:], grd[:])
        nc.sync.dma_start(GHD[:], ghd[:])

        hpool = es.enter_context(tc.tile_pool(name="h", bufs=2 * NCH))
        gates = es.enter_context(tc.tile_pool(name="gates", bufs=3))
        samp = es.enter_context(tc.tile_pool(name="samp", bufs=2))
        stage = es.enter_context(tc.tile_pool(name="stage", bufs=5))
        ps_g = es.enter_context(tc.tile_pool(
            name="psg", bufs=int(os.environ.get("KERNEL_PS_BUFS", "6")),
            space="PSUM"))

        z3 = bass.ts(0, U)   # gate column ranges in R/K
        r3 = bass.ts(1, U)
        h3 = bass.ts(2, U)

        h = []
        for c in range(NCH):
            hc = hpool.tile([U, CW], RD, tag=f"h{c}", name="h0")
            nc.vector.memset(hc[:], 0.0)
            h.append(hc)

        def enc_prefill(xb):
            """K@x prefills: they depend only on the (prefetched) input,
            so they are emitted a step early and run in the PE gap while
            the previous step's gate chain drains."""
            pre = []
            for c in range(NCH):
                x_row = xb[0:1, bass.ts(c, CW)]
                pz = ps_g.tile([U, CW], F32, tag="ps", name="pz")
                pr = ps_g.tile([U, CW], F32, tag="ps", name="pr")
                nc.tensor.matmul(pr[:], K[:, r3], x_row, start=True,
                                 stop=False)
                nc.tensor.matmul(pz[:], K[:, z3], x_row, start=True,
                                 stop=False)
                pre.append((pz, pr))
            return pre

        def gru_mms_enc(c, pre):
            """Encoder R@h matmuls for chunk c, accumulating onto the
            prefilled K@x banks.  r first (the chain head sigma_r waits
            on it), then the h-gate (tt is next), z last (u1's consumers
            are in the tanh shadow / post-tanh)."""
            hc = h[c]
            pz, pr = pre[c]
            ph = ps_g.tile([U, CW], F32, tag="ps", name="ph")
            nc.tensor.matmul(pr[:], R[:, r3], hc[:], start=False, stop=True)
            nc.tensor.matmul(ph[:], R[:, h3], hc[:], start=True, stop=True)
            nc.tensor.matmul(pz[:], R[:, z3], hc[:], start=False, stop=True)
            return pz, pr, ph

        def gru_rh_mms_dec(c):
            """Decoder R@h matmuls for chunk c (issued early: they only
            need the previous step's h, and keep the PE busy while the
            sample chain runs)."""
            hc = h[c]
            pz = ps_g.tile([U, CW], F32, tag="ps", name="pz")
            pr = ps_g.tile([U, CW], F32, tag="ps", name="pr")
            ph = ps_g.tile([U, CW], F32, tag="ps", name="ph")
            nc.tensor.matmul(pr[:], R[:, r3], hc[:], start=True, stop=False)
            nc.tensor.matmul(ph[:], R[:, h3], hc[:], start=True, stop=True)
            nc.tensor.matmul(pz[:], R[:, z3], hc[:], start=True, stop=False)
            return pz, pr, ph

        def gru_ky_mms(c, pz, pr, y):
            """Decoder K@y matmuls for chunk c: z/r accumulate into the
            R@h banks; the h-gate needs its own bank (the reset gate
            scales only the recurrent part)."""
            y_row = y[0:1, bass.ts(c, CW)]
            px = ps_g.tile([U, CW], F32, tag="ps", name="px")
            nc.tensor.matmul(pr[:], K[:, r3], y_row, start=False, stop=True)
            nc.tensor.matmul(px[:], K[:, h3], y_row, start=True, stop=True)
            nc.tensor.matmul(pz[:], K[:, z3], y_row, start=False, stop=True)
            return px

        def gru_tails(pss, bias_z, bias_r, bias_h, xb=None, pxs=None,
                      uxs=None, phss=None):
            """Gate nonlinearities + state update for ALL chunks, in
            cross-chunk phases so no chunk's sigmoids queue behind
            another chunk's tanh on the in-order scalar engine.
            r_ is emitted first (tt, the chain head, needs it);
            h2 = (h - u1*h) + u1*hh, with hz = u1*h on the idle GpSimd
            engine in the tanh shadow.  uxs: precomputed x*K_h tiles
            (encoder); phss: fp16 SBUF copies of the R_h@h PSUM (decoder,
            made during the y-DMA window) letting tt run at 2x."""
            u1s, r_s, uus, fs = [], [], [], []
            for c in range(NCH):
                pz, pr, ph = pss[c]
                r_ = gates.tile([U, CW], RD16, tag=f"r_{c}", name="r_")
                nc.scalar.activation(r_[:], pr[:], AF.Sigmoid, bias=bias_r,
                                     scale=1.0)
                u1 = gates.tile([U, CW], RD16, tag=f"u1_{c}", name="u1")
                nc.scalar.activation(u1[:], pz[:], AF.Sigmoid, bias=bias_z,
                                     scale=-1.0)
                u1s.append(u1)
                r_s.append(r_)
            for c in range(NCH):
                pz, pr, ph = pss[c]
                hc = h[c]
                hrec = phss[c] if phss is not None else ph
                if with_b1h:
                    hb = gates.tile([U, CW], F32, tag=f"hb_{c}", name="hb")
                    nc.vector.tensor_scalar(hb[:], ph[:], GB[:, 3:4], None,
                                            op0=ALU.add)
                    hrec = hb
                tt = gates.tile([U, CW], RD16, tag=f"t_{c}", name="tt")
                nc.vector.tensor_mul(tt[:], r_s[c][:], hrec[:])
                uu = gates.tile([U, CW], RD16, tag=f"u_{c}", name="uu")
                if uxs is not None:
                    nc.vector.tensor_add(uu[:], tt[:], uxs[c][:])
                elif xb is not None:
                    nc.vector.scalar_tensor_tensor(
                        uu[:], xb[:, bass.ts(c, CW)], KC[:, 2:3], tt[:],
                        op0=ALU.mult, op1=ALU.add)
                else:
                    nc.vector.tensor_add(uu[:], tt[:], pxs[c][:])
                uus.append(uu)
                hz = gates.tile([U, CW], RD16, tag=f"hz_{c}", name="hz")
                nc.vector.tensor_mul(hz[:], u1s[c][:], hc[:])
                f = gates.tile([U, CW], RD16, tag=f"f_{c}", name="f")
                nc.vector.tensor_sub(f[:], hc[:], hz[:])
                fs.append(f)
            hhs = []
            for c in range(NCH):
                hh = gates.tile([U, CW], RD16, tag=f"hh_{c}", name="hh")
                nc.scalar.activation(hh[:], uus[c][:], AF.Tanh, bias=bias_h,
                                     scale=1.0)
                hhs.append(hh)
            for c in range(NCH):
                g = gates.tile([U, CW], RD16, tag=f"g_{c}", name="g")
                nc.vector.tensor_mul(g[:], u1s[c][:], hhs[c][:])
                h2 = hpool.tile([U, CW], RD, tag=f"h{c}", name="h2")
                nc.vector.tensor_add(h2[:], fs[c][:], g[:])
                h[c] = h2

        def dense_var(t):
            """Batch-major dense head for step t: per 128-batch slice,
            lhsT = h-slice (stationary), rhs = [W0|W1] columns -> PSUM
            [128, 2*NQ] with batch on partitions.  Then one micro
            sigmoid (resident table)."""
            ps4 = ps_g.tile([U, 2 * NQ + (448 if _FILL_DEC else 0)], F32,
                            tag="ps4", bufs=1, name="ps4")
            for c in range(NCH):
                for j in range(SL):
                    q = SL * c + j
                    nc.tensor.matmul(
                        ps4[:, 2 * q: 2 * q + 2],
                        h[c][:, U * j: U * j + U],
                        WKP[:, 2 * t: 2 * t + 2],
                        start=True, stop=True)
            g4 = samp.tile([U, NQ], F32, tag="g4", name="g4")
            # g = sigmoid(-(v + C + wb1)) = e^{-softplus(v+C+wb1)}
            nc.scalar.activation(g4[:], ps4[:, 1:2*NQ:2], AF.Sigmoid,
                                 bias=CB1B[:, t: t + 1], scale=-1.0)
            return ps4, g4

        def sample_and_out(t, ps4, g4, pss=None, last=False):
            """DVE fastlog softplus + reparameterized sample + outputs.
            ln(g) = (e-127)*ln2 + poly(mantissa) via bitcast tricks; the
            sample m = (1e-5 + 0.05*sp)*eps; y = m + loc_raw (wb0 rides
            the next step's gate biases).  One small DMA flips y[128,NQ]
            into the row form the K@y matmuls need."""
            bb = g4[:].bitcast(U32)
            e_ = samp.tile([U, NQ], U32, tag="e_", name="e_")
            nc.vector.tensor_scalar(e_[:], bb, 23, None,
                                    op0=ALU.logical_shift_right)
            f_ = samp.tile([U, NQ], U32, tag="f_", name="f_")
            nc.vector.tensor_scalar(f_[:], bb, 0x007FFFFF, 0x3F800000,
                                    op0=ALU.bitwise_and, op1=ALU.bitwise_or)
            ff = f_[:].bitcast(F32)
            p1 = samp.tile([U, NQ], F32, tag="p1", name="p1")
            nc.vector.tensor_scalar(p1[:], ff, _C3, _C2, op0=ALU.mult,
                                    op1=ALU.add)
            p2 = samp.tile([U, NQ], F32, tag="p2", name="p2")
            nc.vector.tensor_tensor(p2[:], p1[:], ff, op=ALU.mult)
            p3 = samp.tile([U, NQ], F32, tag="p3", name="p3")
            nc.vector.scalar_tensor_tensor(p3[:], p2[:], _C1, ff,
                                           op0=ALU.add, op1=ALU.mult)
            p4 = samp.tile([U, NQ], F32, tag="p4", name="p4")
            nc.vector.scalar_tensor_tensor(p4[:], e_[:], _LN2, p3[:],
                                           op0=ALU.mult, op1=ALU.add)
            # outputs (off the critical chain; host unpermutes batch-major)
            loc4 = samp.tile([U, NQ], F32, tag="loc4", name="loc4")
            nc.vector.tensor_scalar(loc4[:], ps4[:, 0:2*NQ:2], WB0B[:, t: t + 1],
                                    None, op0=ALU.add)
            nc.sync.dma_start(out_bm[t: t + 1, 0:1], loc4[:])
            sc4 = samp.tile([U, NQ], F32, tag="sc4", name="sc4")
            nc.vector.tensor_scalar(sc4[:], p4[:], -OP_SCALE, _SAMP_S1,
                                    op0=ALU.mult, op1=ALU.add)
            nc.sync.dma_start(out_bm[t: t + 1, 1:2], sc4[:])
            if last:
                return None
            ep4 = stage.tile([U, NQ], F32, tag="eps", name="ep4")
            nc.sync.dma_start(ep4[:], eps_bm[t: t + 1])
            m4 = samp.tile([U, NQ], F32, tag="m4", name="m4")
            nc.vector._custom_dve(
                AFFINE_MUL_REDUCE, out=m4[:], in0=p4[:], in1=ep4[:],
                s0=-OP_SCALE, s1=_SAMP_S1)
            y4 = samp.tile([U, NQ], RD, tag="y4", name="y4")
            nc.vector.tensor_add(y4[:], m4[:], ps4[:, 0:2*NQ:2])
            # y4 [128, NQ] -> per-chunk PE transpose -> [SL, 128] PSUM
            # (chunk c at partition base 32c), scalar-copy to SBUF, then
            # a contiguous DMA lands each chunk's y row; chunk 0's K@y
            # starts while chunk 1's DMA is still in flight.
            yt = ps_g.tile([32 * (NCH - 1) + SL, U], RD, tag="yt", bufs=1,
                           name="yt")
            y = samp.tile([1, BC], RD, tag="y", name="y")
            for c in range(NCH):
                p0 = 32 * c
                nc.tensor.matmul(yt[p0:p0 + SL, :],
                                 y4[:, SL * c: SL * c + SL], IDT[:],
                                 is_transpose=True, skip_group_check=True)
                ys = samp.tile([SL, U], RD, tag=f"ys{c}", name="ys")
                nc.scalar.copy(ys[:], yt[p0:p0 + SL, :])
                nc.sync.dma_start(y[0:1, bass.ts(c, CW)], ys[:])
            # scratch matmuls into the unused tail of the ps4 bank keep
            # the PE streaming while it waits for the y DMA
            for _ in range(_FILL_DEC):
                nc.tensor.matmul(ps4[:, 2 * NQ: 2 * NQ + 448], R[:, z3],
                                 h[0][:, 0:448], start=True, stop=True)
            return y, None

        # ---- encoder: 48 GRU steps over the input sequence ----
        xbs = {}

        def enc_xb(t):
            xb = stage.tile([U, BC], RD, tag="xb", name="xb")
            nc.sync.dma_start(xb[:], x_seq[t: t + 1, :].partition_broadcast(U))
            return xb

        xbs[0] = enc_xb(0)
        pre = enc_prefill(xbs[0])
        for t in range(T_ENC):
            xb = xbs.pop(t)
            ps = [gru_mms_enc(c, pre) for c in range(NCH)]
            if t + 1 < T_ENC:
                xbs[t + 1] = enc_xb(t + 1)
                pre = enc_prefill(xbs[t + 1])
            gru_tails(ps, GB[:, 0:1], GB[:, 1:2], GB[:, 2:3], xb=xb)

        # ---- decoder: dense head + 27 sampled feedback GRU steps ----
        ps4, g4 = dense_var(0)
        for t in range(1, GAMMA):
            j = t - 1
            # R@h first: they only need the previous h and keep the PE
            # busy under the sample chain.
            ps = [gru_rh_mms_dec(c) for c in range(NCH)]
            y, phss = sample_and_out(j, ps4, g4, pss=ps)
            pxs = [gru_ky_mms(c, ps[c][0], ps[c][1], y) for c in range(NCH)]
            gru_tails(ps, GZD[:, j:j + 1], GRD[:, j:j + 1],
                      GHD[:, j:j + 1], pxs=pxs, phss=phss)
            ps4, g4 = dense_var(t)
        sample_and_out(GAMMA - 1, ps4, g4, last=True)

    nc.compile()
    return nc


def _host_prep(inputs, gru_kernel, gru_rec_kernel, gru_bias, dv_loc, dv_rho,
               dv_eps, samp_eps):
    """Host-side input preprocessing -> per-core input maps."""
    inputs = np.asarray(inputs, np.float32)
    B = inputs.shape[0]
    assert B == B_FULL, f"kernel compiled for B={B_FULL}, got {B}"
    xT = _cvt(inputs[:, :T_ENC, 0].T)                          # [48, B]
    epsT = np.ascontiguousarray(np.asarray(samp_eps, np.float32)[:, :, 0])  # [27, B]

    gru_kernel = np.asarray(gru_kernel, np.float32)
    gru_bias = np.asarray(gru_bias, np.float32)
    b0, b1 = gru_bias[0], gru_bias[1]
    kz = gru_kernel[0, 0:U]
    kr = gru_kernel[0, U:2 * U]
    kh = gru_kernel[0, 2 * U:3 * U]
    gb = np.zeros((U, 4), np.float32)
    gb[:, 0] = -(b0[0:U] + b1[0:U])
    gb[:, 1] = b0[U: 2 * U] + b1[U: 2 * U]
    gb[:, 2] = b0[2 * U: 3 * U]
    gb[:, 3] = b1[2 * U: 3 * U]

    dv_loc = np.asarray(dv_loc, np.float32)
    dv_rho = np.asarray(dv_rho, np.float32)
    dv_eps = np.asarray(dv_eps, np.float32)
    scale_q = np.float32(1e-5) + np.float32(Q_SCALE) * np.logaddexp(
        np.float32(C_SP) + dv_rho, np.float32(0.0), dtype=np.float32)
    w_all = dv_loc[None, :] + scale_q[None, :] * dv_eps        # [28, 258]
    wkp = np.ascontiguousarray(
        w_all[:, : 2 * U].reshape(GAMMA, U, 2).transpose(1, 0, 2)
        .reshape(U, 2 * GAMMA))
    wb0 = w_all[:, 2 * U]                                      # [28]
    cb1 = -(np.float32(C_SP) + w_all[:, 2 * U + 1])            # [28]
    wb0b = np.ascontiguousarray(
        np.broadcast_to(wb0[None, :], (U, GAMMA)), np.float32)
    cb1b = np.ascontiguousarray(
        np.broadcast_to(cb1[None, :], (U, GAMMA)), np.float32)

    # decoder per-step gate biases with K_g*wb0 folded in (gru step t
    # consumes the dense head of step j=t-1 -> columns indexed by j)
    wb0d = wb0[: GAMMA - 1]
    gzd = -((b0[0:U] + b1[0:U])[:, None] + np.outer(kz, wb0d))  # [U, 27]
    grd = (b0[U:2 * U] + b1[U:2 * U])[:, None] + np.outer(kr, wb0d)
    ghd = b0[2 * U:3 * U][:, None] + np.outer(kh, wb0d)

    shared = {
        "r_w": _cvt(gru_rec_kernel),
        "k_w": _cvt(gru_kernel),
        "k_col": np.ascontiguousarray(gru_kernel.reshape(3, U).T),
        "wkp": _cvt(wkp),
        "wb0b": wb0b,
        "cb1b": cb1b,
        "gb": gb,
        "gzd": np.ascontiguousarray(gzd, np.float32),
        "grd": np.ascontiguousarray(grd, np.float32),
        "ghd": np.ascontiguousarray(ghd, np.float32),
        "idt": _cvt(np.eye(U, dtype=np.float32)),
    }
    in_maps = []
    for c in range(N_CORES):
        sl = slice(c * BC, (c + 1) * BC)
        # eps batch-major: eps_bm[t, p, q] = eps[t, 512*(q//SL)+128*(q%SL)+p]
        e = epsT[:, sl].reshape(GAMMA - 1, NCH, SL, U)
        e = np.ascontiguousarray(e.transpose(0, 3, 1, 2).reshape(
            GAMMA - 1, U, NQ))
        in_maps.append(
            dict(
                shared,
                x_seq=np.ascontiguousarray(xT[:, sl]),
                eps_bm=e,
            )
        )
    return in_maps, bool(np.any(gb[:, 3] != 0.0))


def _get_nc(with_b1h=False):
    key = ("nc", with_b1h)
    if key not in _CACHE:
        _CACHE[key] = _build_program(with_b1h)
    return _CACHE[key]


def _unpack_out(fm):
    """[GAMMA, 2, U, NQ] batch-major -> [BC, GAMMA, 2]."""
    a = fm.transpose(2, 3, 0, 1)                      # [p, q, t, k]
    a = a.reshape(U, NCH, SL, GAMMA, 2)               # [p, c, j, t, k]
    return a.transpose(1, 2, 0, 3, 4).reshape(BC, GAMMA, 2)


def run(inputs_dict, trace=False, trace_kwargs=None):
    in_maps, with_b1h = _host_prep(**inputs_dict)
    nc = _get_nc(with_b1h)
    res = run_bass_kernel_spmd(
        nc, in_maps, list(range(N_CORES)), trace=trace,
        **(trace_kwargs or {}),
    )
    _CACHE["last_results"] = res
    out = np.empty((B_FULL, GAMMA, 2), np.float32)
    for c in range(N_CORES):
        out[c * BC: (c + 1) * BC] = _unpack_out(res.results[c]["out_bm"])
    return out


def kernel(**inputs):
    return run(inputs, trace=bool(os.environ.get("KERNEL_TRACE")))
